# revision 41
# baseline (speedup 1.0000x reference)
"""Trainium2 Bass kernel for nn_DirectedODRLoss (retrieval_knn).

Math (B=4096, D=256, k=25, scales (1,2,3)):
    dist²(i,j) = |f_i|² + |f_j|² − 2 f_i·f_j ;  y := −dist²  (f32 strips;
        bf16 Gram GEMM; −|f_i|² per-partition scalar and −|f_j|² f32 row
        strip folded into one DVE scalar_tensor_tensor on the PSUM drain)
    topk: per row, candidates = top-8 of each of 16 256-wide chunks (16
        f32 max8 passes) → top-25 of the 128 candidates via max8 +
        match_replace;  τ_i := 25th largest y;  σ_i = mean(sqrt(−y+eps))
    mutual knn mask:  y symmetric ⇒ mutual(i,j) = [y_ij ≥ max(τ_i, τ_j)]
    drop = [max(τ_i,τ_j) > y] | [lab_j < lab_i];  e = y·rσ_j (fp16)
    Wn = exp(rσ_i·(e − 1024·drop))   (drop ⇒ exp(≤ −45/σ²) ⇒ 0 in fp16)
    S_i = ΣWn + 1,  P = (Wn + dsel·I)/S  (f32 in place; diag via a
        per-core one-hot dsel over column blocks), quantized to fp8 e4m3
    loss = (1/B)(C1 + C2/2 + C3/3) with
        C1 = <Wn/S, pen>,  C2 = <A, pen>,  C3 = <U, pen>,
        A = P²  (computed transposed: Aᵀ chunks, lhsT = raw pfull column
        strips, rhs = SBUF-resident P_shardᵀ),  U = A·P (Uᵀ likewise),
        pen_ij = relu(s_i − s_j) (fp16).

Sharding: rows split across 8 cores. P all-gathered in fp8; both B³ GEMMs
run in fp8 DoubleRow mode (0.5 cyc/row); Aᵀ/Uᵀ chunks consumed against
recomputed penᵀ tiles via fused scalar_tensor_tensor accumulations.
Final scalars all-reduced.
"""

import ml_dtypes
import numpy as np

import concourse.bacc as bacc
import concourse.bass as bass
import concourse.mybir as mybir
import concourse.tile as tile
from concourse.masks import make_identity

F32 = mybir.dt.float32
F16 = mybir.dt.float16
BF16 = mybir.dt.bfloat16
F8 = mybir.dt.float8e4
AX = mybir.AxisListType
OP = mybir.AluOpType
ACT = mybir.ActivationFunctionType
DRM = mybir.MatmulPerfMode.DoubleRow

EPS = 1e-8
KNN = 25
DROP_SHIFT = 1024.0
NEG_FILL = -1e30


def build_program(B=4096, D=256, NC=8):
    P = 128
    R = B // NC            # rows per core (512)
    NMT = R // P           # row tiles per core (4)
    TN = R                 # column tile (512)
    NNT = B // TN          # column tiles (8)
    KC = B // P            # 128-row chunks of B (32)
    KC2 = KC // 2          # DoubleRow steps (16)
    GK = D // P            # contraction chunks for the Gram GEMM (2)
    TC = 16                # topk candidate chunks per row
    CW = B // TC           # chunk width (256)

    nc = bacc.Bacc("TRN2", target_bir_lowering=False, debug=False,
                   num_devices=NC)

    # ---- I/O ----------------------------------------------------------------
    ft2 = nc.dram_tensor("ft2", [D, R], BF16, kind="ExternalInput")    # 2·F_shardᵀ
    ftf = nc.dram_tensor("ftf", [D, B], BF16, kind="ExternalInput")    # Fᵀ (full)
    fsh = nc.dram_tensor("fsh", [R, D], BF16, kind="ExternalInput")    # F shard
    srow = nc.dram_tensor("srow", [1, B], F32, kind="ExternalInput")   # scores
    srow16 = nc.dram_tensor("srow16", [1, B], F16, kind="ExternalInput")
    smyrow16 = nc.dram_tensor("smyrow16", [1, R], F16, kind="ExternalInput")
    scols = nc.dram_tensor("scols", [P, NMT], F32, kind="ExternalInput")
    lrow16 = nc.dram_tensor("lrow16", [1, B], F16, kind="ExternalInput")
    lcols = nc.dram_tensor("lcols", [P, NMT], F32, kind="ExternalInput")
    dsel = nc.dram_tensor("dsel", [1, NNT], F32, kind="ExternalInput")  # one-hot(rank)
    loss_out = nc.dram_tensor("loss", [1, 1], F32, kind="ExternalOutput")

    # ---- internal DRAM ------------------------------------------------------
    # P is all-gathered in two row-halves so the first AG overlaps the
    # second half of stage W. Half h of pfull holds rank-major blocks of
    # 256 rows: global row c*R + h*256 + q*128 + p ↔ pfull_h[c*256+q*128+p].
    pn_half = [nc.dram_tensor(f"pn_half{h}", [R // 2, B], F8) for h in range(2)]
    pfull_h = [nc.dram_tensor(f"pfull{h}", [NC * R // 2, B], F8,
                              addr_space="Shared") for h in range(2)]
    stats_in = nc.dram_tensor("stats_in", [1, 2 * R], F32)
    stats_out = nc.dram_tensor("stats_out", [NC, 2 * R], F32, addr_space="Shared")
    d_dram = nc.dram_tensor("d_dram", [1, R], F32)
    d_out = nc.dram_tensor("d_out", [NC, R], F32, addr_space="Shared")
    red_in = nc.dram_tensor("red_in", [1, 8], F32)
    red_out = nc.dram_tensor("red_out", [1, 8], F32, addr_space="Shared")

    rg = [list(range(NC))]

    with tile.TileContext(nc) as tc:
        with (
            tc.tile_pool(name="const", bufs=1) as constp,
            tc.tile_pool(name="io", bufs=2) as iop,
            tc.tile_pool(name="pnp", bufs=1) as pnp,
            tc.tile_pool(name="big", bufs=1) as bigp,
            tc.tile_pool(name="strip", bufs=1) as stripp,
            tc.tile_pool(name="cols", bufs=1) as colp,
            tc.tile_pool(name="work", bufs=2) as workp,
            tc.tile_pool(name="psum", bufs=1, space="PSUM") as psump,
        ):
            def ps_tile(tag, shape=None, dtype=F32):
                return psump.tile(shape or [P, TN], dtype, tag=tag, name=tag)

            # Gram operands (bf16, cached in SBUF, reused per mt tile)
            ft2_sb = constp.tile([P, GK * R], BF16, tag="ft2_sb")
            for g in range(GK):
                nc.sync.dma_start(ft2_sb[:, g * R:(g + 1) * R],
                                  ft2[g * P:(g + 1) * P, :])
            ftf_sb = bigp.tile([P, GK * B], BF16, tag="fbuf", name="ftf_sb")
            for g in range(GK):
                nc.sync.dma_start(ftf_sb[:, g * B:(g + 1) * B],
                                  ftf[g * P:(g + 1) * P, :])

            # ============ stage A: squared norms ============================
            # full |f|²: square ftf (f32 out), column-reduce via ones-matmul
            # broadcast to all 128 partitions (lhsT = all-ones matrix)
            sq2 = bigp.tile([P, GK * B], F32, tag="ybuf", name="sq2")
            for g in range(GK):
                nc.scalar.activation(sq2[:, g * B:(g + 1) * B],
                                     ftf_sb[:, g * B:(g + 1) * B], ACT.Square)
            ones_m = constp.tile([P, P], F32, tag="ones_m")
            nc.vector.memset(ones_m[:], 1.0)
            sqj_b = stripp.tile([P, B], F32, tag="sqj_b")
            for ntb in range(NNT):
                psq = ps_tile(f"pa{ntb % 4}")
                for g in range(GK):
                    nc.tensor.matmul(
                        psq[:], ones_m[:],
                        sq2[:, g * B + ntb * TN: g * B + (ntb + 1) * TN],
                        start=(g == 0), stop=(g == GK - 1))
                nc.scalar.activation(sqj_b[:, ntb * TN:(ntb + 1) * TN],
                                     psq[:], ACT.Copy)
            # shard |f|² in column layout (per-partition scalars, negated)
            sqcs = colp.tile([P, NMT], F32, tag="sqcs")
            fshb = iop.tile([P, NMT * D], BF16, tag="fblk", name="fshb")
            nc.sync.dma_start(
                fshb[:].rearrange("p (t d) -> p t d", t=NMT),
                bass.AP(fsh, 0, [[D, P], [P * D, NMT], [1, D]]))
            for q in range(NMT):
                scr = workp.tile([P, D], F32, tag="sqscr")
                nc.scalar.activation(scr[:], fshb[:, q * D:(q + 1) * D],
                                     ACT.Square, accum_out=sqcs[:, q:q + 1])
            sqcsr = colp.tile([P, NMT], F32, tag="sqcsr")
            nc.vector.tensor_scalar(sqcsr[:], sqcs[:], -1.0, None, OP.mult)

            # misc broadcast loads
            dsel_c = colp.tile([P, NNT], F32, tag="dsel_c")
            nc.sync.dma_start(dsel_c[:], bass.AP(dsel, 0, [[0, P], [1, NNT]]))
            lab_c = colp.tile([P, NMT], F32, tag="lab_c")
            s_c = colp.tile([P, NMT], F32, tag="s_c")
            nc.sync.dma_start(lab_c[:], lcols[:, :])
            nc.sync.dma_start(s_c[:], scols[:, :])
            lab_b = stripp.tile([P, B], F16, tag="lab_b")
            s_b = stripp.tile([P, B], F16, tag="s_b")
            smy_b = constp.tile([P, R], F16, tag="smy_b")
            nc.sync.dma_start(lab_b[:], bass.AP(lrow16, 0, [[0, P], [1, B]]))
            nc.sync.dma_start(s_b[:], bass.AP(srow16, 0, [[0, P], [1, B]]))
            nc.sync.dma_start(smy_b[:], bass.AP(smyrow16, 0, [[0, P], [1, R]]))
            sfc = colp.tile([P, KC], F32, tag="sfc")       # −s_g, col layout
            nc.sync.dma_start(sfc[:], bass.AP(srow, 0, [[1, P], [P, KC]]))
            nc.vector.tensor_scalar(sfc[:], sfc[:], -1.0, None, OP.mult)

            ident16 = constp.tile([P, P], F16, tag="ident16")
            make_identity(nc, ident16[:])
            ndsel_c = colp.tile([P, NNT], F32, tag="ndsel_c")
            nc.vector.tensor_scalar(ndsel_c[:], dsel_c[:], -1.0, None, OP.mult)
            eps_c = constp.tile([P, 1], F32, tag="eps_c")
            nc.vector.memset(eps_c[:], EPS)

            # ============ stage B: Gram → y (f32) + chunked topk ============
            y_all = bigp.tile([P, NMT * B], F32, tag="ybuf", name="y_all")
            cands = colp.tile([P, TC * 8], F32, tag="cands")
            vals = colp.tile([P, 32], F32, tag="vals")
            yt_cols = colp.tile([P, NMT], F32, tag="yt_cols")
            rs_cols = colp.tile([P, NMT], F32, tag="rs_cols")
            ssum = colp.tile([P, NMT], F32, tag="ssum")
            for mt in range(NMT):
                gps = [ps_tile(f"pa{ntb}") for ntb in range(NNT)]
                for g in range(GK):
                    for ntb in range(NNT):
                        nc.tensor.matmul(
                            gps[ntb][:],
                            ft2_sb[:, g * R + mt * P: g * R + (mt + 1) * P],
                            ftf_sb[:, g * B + ntb * TN: g * B + (ntb + 1) * TN],
                            start=(g == 0), stop=(g == GK - 1))
                ys = y_all[:, mt * B:(mt + 1) * B]
                for ntb in range(NNT):
                    # y = (gram − |f_i|²) − |f_j|²   (f32 out)
                    nc.vector.scalar_tensor_tensor(
                        ys[:, ntb * TN:(ntb + 1) * TN], gps[ntb][:],
                        sqcsr[:, mt:mt + 1], sqj_b[:, ntb * TN:(ntb + 1) * TN],
                        op0=OP.add, op1=OP.subtract)
                # candidates: top-8 of each 256-wide chunk
                for t in range(TC):
                    nc.vector.max(out=cands[:, t * 8:(t + 1) * 8],
                                  in_=ys[:, t * CW:(t + 1) * CW])
                # top-25 of the 128 candidates
                ca = workp.tile([P, TC * 8], F32, tag="ca", name="ca")
                cb = workp.tile([P, TC * 8], F32, tag="cb", name="cb")
                nc.vector.max(out=vals[:, 0:8], in_=cands[:])
                nc.vector.match_replace(out=ca[:], in_to_replace=vals[:, 0:8],
                                        in_values=cands[:], imm_value=NEG_FILL)
                nc.vector.max(out=vals[:, 8:16], in_=ca[:])
                nc.vector.match_replace(out=cb[:], in_to_replace=vals[:, 8:16],
                                        in_values=ca[:], imm_value=NEG_FILL)
                nc.vector.max(out=vals[:, 16:24], in_=cb[:])
                nc.vector.match_replace(out=ca[:], in_to_replace=vals[:, 16:24],
                                        in_values=cb[:], imm_value=NEG_FILL)
                nc.vector.max(out=vals[:, 24:32], in_=ca[:])
                # τ_i = 25th largest y
                nc.vector.tensor_copy(yt_cols[:, mt:mt + 1], vals[:, 24:25])
                # σ_i = mean sqrt(max(d,0)+eps) over 25 NN;  d = −y
                c25 = workp.tile([P, KNN], F32, tag="c25")
                nc.vector.tensor_scalar(c25[:], vals[:, 0:KNN], 0.0, None,
                                        OP.min)
                s25 = workp.tile([P, KNN], F32, tag="s25")
                nc.scalar.activation(s25[:], c25[:], ACT.Sqrt,
                                     bias=eps_c[:, 0:1], scale=-1.0,
                                     accum_out=ssum[:, mt:mt + 1])
            nc.vector.reciprocal(rs_cols[:], ssum[:])
            nc.vector.tensor_scalar(rs_cols[:], rs_cols[:], float(KNN), None,
                                    OP.mult)

            # stats all-gather: flat per-rank [τ(R) ++ rσ(R)] (f32), both in
            # shard-row order g_local = c*128 + p.
            nc.sync.dma_start(bass.AP(stats_in, 0, [[1, P], [P, NMT]]),
                              yt_cols[:])
            nc.sync.dma_start(bass.AP(stats_in, R, [[1, P], [P, NMT]]),
                              rs_cols[:])
            nc.gpsimd.collective_compute(
                "AllGather", OP.bypass, replica_groups=rg,
                ins=[stats_in.ap().opt()], outs=[stats_out.ap().opt()])

            def stat_bcast_ap(off):
                return bass.AP(stats_out, off, [[0, P], [2 * R, NC], [1, R]])

            yt_b = stripp.tile([P, B], F32, tag="yt_b")
            nc.sync.dma_start(yt_b[:].rearrange("a (r q) -> a r q", r=NC),
                              stat_bcast_ap(0))
            # rσ strip: stage f32 into sqj_b's buffer (dead after stage B),
            # convert to fp16
            rs_f32 = stripp.tile([P, B], F32, tag="sqj_b", name="rs_f32")
            nc.sync.dma_start(rs_f32[:].rearrange("a (r q) -> a r q", r=NC),
                              stat_bcast_ap(R))
            rs_b = stripp.tile([P, B], F16, tag="rs_b")
            nc.vector.tensor_copy(rs_b[:], rs_f32[:])

            # ============ stage W: Wn, S, P, C1 =============================
            srcols = colp.tile([P, NMT * NNT], F32, tag="srcols")
            c1cols = colp.tile([P, NMT * NNT], F32, tag="c1cols")
            invS = colp.tile([P, NMT], F32, tag="invS")
            Scol = colp.tile([P, NMT], F32, tag="Scol")
            pn8 = [pnp.tile([P, B], F8, tag="pn8", name=f"pn8_{mt}")
                   for mt in range(NMT)]
            for mt in range(NMT):
                ys = y_all[:, mt * B:(mt + 1) * B]
                for ntb in range(NNT):
                    yt_t = ys[:, ntb * TN:(ntb + 1) * TN]
                    # drop = [max(τ_j, τ_i) > y] | [lab_j < lab_i]
                    drop1 = workp.tile([P, TN], F16, tag="w1", name="drop1")
                    nc.vector.scalar_tensor_tensor(
                        drop1[:], yt_b[:, ntb * TN:(ntb + 1) * TN],
                        yt_cols[:, mt:mt + 1], yt_t,
                        op0=OP.max, op1=OP.is_gt)
                    drop = workp.tile([P, TN], F16, tag="w2", name="drop")
                    nc.vector.scalar_tensor_tensor(
                        drop[:], lab_b[:, ntb * TN:(ntb + 1) * TN],
                        lab_c[:, mt:mt + 1], drop1[:],
                        op0=OP.is_lt, op1=OP.max)
                    # e = y·rσ_j (fp16); e2 = e − 1024·drop
                    e = workp.tile([P, TN], F16, tag="w3", name="e")
                    nc.vector.tensor_tensor(
                        e[:], yt_t, rs_b[:, ntb * TN:(ntb + 1) * TN], OP.mult)
                    e2 = workp.tile([P, TN], F16, tag="w1", name="e2")
                    nc.vector.scalar_tensor_tensor(
                        e2[:], drop[:], -DROP_SHIFT, e[:],
                        op0=OP.mult, op1=OP.add)
                    # Wn = exp(rσ_i·e2), written over the f32 y strip,
                    # accum → ΣWn
                    nc.scalar.activation(yt_t, e2[:], ACT.Exp,
                                         scale=rs_cols[:, mt:mt + 1],
                                         accum_out=srcols[:, mt * NNT + ntb:
                                                          mt * NNT + ntb + 1])

                # S = ΣWn + 1 ;  invS = 1/S
                nc.vector.reduce_sum(Scol[:, mt:mt + 1],
                                     srcols[:, mt * NNT:(mt + 1) * NNT],
                                     axis=AX.X)
                nc.vector.tensor_scalar(Scol[:, mt:mt + 1], Scol[:, mt:mt + 1],
                                        1.0, None, OP.add)
                nc.vector.reciprocal(invS[:, mt:mt + 1], Scol[:, mt:mt + 1])
                # Q = offdiag(P) = (Wn − dsel·I)/S  (diagonal handled exactly
                # via d = 2/S in the GEMM correction terms)
                for ntb in range(NNT):
                    dslice = ys[:, ntb * TN + mt * P: ntb * TN + (mt + 1) * P]
                    nc.vector.scalar_tensor_tensor(
                        dslice, ident16[:], ndsel_c[:, ntb:ntb + 1], dslice,
                        op0=OP.mult, op1=OP.add)
                    nc.vector.tensor_scalar(ys[:, ntb * TN:(ntb + 1) * TN],
                                            ys[:, ntb * TN:(ntb + 1) * TN],
                                            invS[:, mt:mt + 1], None, OP.mult)
                    # C1 partial: Σ Q·pen = Σ Wn·pen/S  (pen_ii = 0):
                    # gpsimd multiply + scalar accumulate-copy
                    pen = workp.tile([P, TN], F16, tag="w3", name="pen")
                    nc.scalar.activation(pen[:], s_b[:, ntb * TN:(ntb + 1) * TN],
                                         ACT.Relu, bias=s_c[:, mt:mt + 1],
                                         scale=-1.0)
                    prod = workp.tile([P, TN], F16, tag="w1", name="prod")
                    nc.gpsimd.tensor_tensor(prod[:],
                                            ys[:, ntb * TN:(ntb + 1) * TN],
                                            pen[:], OP.mult)
                    junk = workp.tile([P, TN], F16, tag="w2", name="junk")
                    nc.scalar.activation(junk[:], prod[:], ACT.Copy,
                                         accum_out=c1cols[:, mt * NNT + ntb:
                                                          mt * NNT + ntb + 1])
                nc.scalar.activation(pn8[mt][:], ys, ACT.Copy)
                nc.sync.dma_start(
                    bass.AP(pn_half[mt // 2], (mt % 2) * P * B,
                            [[B, P], [1, B]]), pn8[mt][:])
                if mt % 2 == 1:
                    nc.gpsimd.collective_compute(
                        "AllGather", OP.bypass, replica_groups=rg,
                        ins=[pn_half[mt // 2].ap().opt()],
                        outs=[pfull_h[mt // 2].ap().opt()])

            # C1 finalize (1/S already folded in via the Q strip)
            c1v = colp.tile([P, 1], F32, tag="c1v")
            nc.vector.reduce_sum(c1v[:], c1cols[:], axis=AX.X)

            # ============ all-gather of diag d = 2/S (f32) ==================
            dloc = colp.tile([P, NMT], F32, tag="dloc")
            nc.vector.tensor_scalar(dloc[:], invS[:], 2.0, None, OP.mult)
            nc.sync.dma_start(bass.AP(d_dram, 0, [[1, P], [P, NMT]]), dloc[:])
            nc.gpsimd.collective_compute(
                "AllGather", OP.bypass, replica_groups=rg,
                ins=[d_dram.ap().opt()], outs=[d_out.ap().opt()])
            # d in column layout over full B, d² gated by the diag one-hot,
            # and shard d broadcast along the free axis
            d_cols = colp.tile([P, KC], F32, tag="d_cols")
            nc.sync.dma_start(d_cols[:], bass.AP(d_out, 0, [[1, P], [P, KC]]))
            d2g = colp.tile([P, KC], F32, tag="d2g")
            nc.vector.tensor_tensor(d2g[:], d_cols[:], d_cols[:], OP.mult)
            for ntb in range(NNT):
                nc.vector.tensor_scalar(
                    d2g[:, ntb * NMT:(ntb + 1) * NMT],
                    d2g[:, ntb * NMT:(ntb + 1) * NMT],
                    dsel_c[:, ntb:ntb + 1], None, OP.mult)
            d_strip = constp.tile([P, R], F32, tag="d_strip")
            nc.sync.dma_start(d_strip[:], bass.AP(d_dram, 0, [[0, P], [1, R]]))

            # ============ P_shardᵀ (fp8, SBUF) via PE transposes ============
            # transpose the f32 P strips (2 cyc/row), cast fp8 on psum drain
            idf32 = constp.tile([P, P], F32, tag="idf32")
            make_identity(nc, idf32[:])
            psT = bigp.tile([P, KC * R], F8, tag="ptbuf")
            for ntb in range(NNT):
                for u in range(4):
                    ptb = ps_tile(f"pa{(ntb * 4 + u) % 4}", dtype=F32)
                    for mt in range(NMT):
                        nc.tensor.transpose(
                            ptb[:, mt * P:(mt + 1) * P],
                            y_all[:, mt * B + ntb * TN + u * P:
                                  mt * B + ntb * TN + (u + 1) * P],
                            idf32[:])
                    nc.scalar.activation(
                        psT[:, (ntb * 4 + u) * R:(ntb * 4 + u + 1) * R],
                        ptb[:], ACT.Copy)

            # ============ GEMM1: Aᵀ chunks = (P²)ᵀ, consume C2, cast fp8 ====
            c2cols = colp.tile([P, KC], F32, tag="c2cols")
            c3cols = colp.tile([P, KC], F32, tag="c3cols")
            aT = bigp.tile([P, KC * R], F8, tag="atbuf")

            def lhs_strip_load(cbi, phase):
                # ls[p, j, m] with global row-block j = c*4 + q; half h=q//2
                # holds j ≡ 2h, 2h+1 (mod 4) at rows c*256 + (q%2)*128 + p.
                ls = iop.tile([P, KC * P], F8, tag="lhs_strip",
                              name=f"ls{phase}_{cbi}")
                ls4 = ls[:].rearrange("p (c u m) -> p c u m", c=NC, u=4)
                for h in range(2):
                    for u in range(2):
                        nc.sync.dma_start(
                            ls4[:, :, 2 * h + u, :],
                            bass.AP(pfull_h[h], cbi * P + u * P * B,
                                    [[B, P], [2 * P * B, NC], [1, P]]))
                return ls

            def pen_t_tile(cbi):
                # penᵀ chunk [g-block, i]: relu(s_i − s_g)
                pt = workp.tile([P, TN], F16, tag="w3", name=f"pent{cbi}")
                nc.scalar.activation(pt[:], smy_b[:], ACT.Relu,
                                     bias=sfc[:, cbi:cbi + 1], scale=1.0)
                return pt

            for cb in range(KC):
                ls = lhs_strip_load(cb, 0)
                pa = ps_tile(f"pa{cb % 4}")
                for kb in range(KC2):
                    nc.tensor.matmul(
                        pa[:],
                        ls[:, 2 * kb * P:(2 * kb + 2) * P]
                        .rearrange("p (k m) -> p k m", k=2),
                        psT[:, 2 * kb * R:(2 * kb + 2) * R]
                        .rearrange("p (k n) -> p k n", k=2),
                        start=(kb == 0), stop=(kb == KC2 - 1), perf_mode=DRM)
                # diagonal corrections:
                # Aᵀ = (Q8²)ᵀ + d_g'·Q8ᵀ + Q8ᵀ·d_i + dsel·diag(d²)
                qt = psT[:, cb * R:(cb + 1) * R]
                nc.vector.scalar_tensor_tensor(
                    pa[:], qt, d_cols[:, cb:cb + 1], pa[:],
                    op0=OP.mult, op1=OP.add)
                tdi = workp.tile([P, TN], F32, tag="sqscr", name="tdi")
                nc.vector.tensor_tensor(tdi[:], qt, d_strip[:], OP.mult)
                nc.vector.tensor_tensor(pa[:], pa[:], tdi[:], OP.add)
                dgo = (cb % NMT) * P
                nc.vector.scalar_tensor_tensor(
                    pa[:, dgo:dgo + P], ident16[:], d2g[:, cb:cb + 1],
                    pa[:, dgo:dgo + P], op0=OP.mult, op1=OP.add)
                pent = pen_t_tile(cb)
                junk = workp.tile([P, TN], F16, tag="w1", name="junkA")
                nc.vector.scalar_tensor_tensor(
                    junk[:], pa[:], 1.0, pent[:], op0=OP.mult, op1=OP.mult,
                    accum_out=c2cols[:, cb:cb + 1])
                nc.scalar.activation(aT[:, cb * R:(cb + 1) * R], pa[:],
                                     ACT.Copy)

            # ============ GEMM2: Uᵀ chunks = (A·P)ᵀ, consume C3 =============
            for cb in range(KC):
                ls = lhs_strip_load(cb, 1)
                pa = ps_tile(f"pa{4 + cb % 4}")
                for kb in range(KC2):
                    nc.tensor.matmul(
                        pa[:],
                        ls[:, 2 * kb * P:(2 * kb + 2) * P]
                        .rearrange("p (k m) -> p k m", k=2),
                        aT[:, 2 * kb * R:(2 * kb + 2) * R]
                        .rearrange("p (k n) -> p k n", k=2),
                        start=(kb == 0), stop=(kb == KC2 - 1), perf_mode=DRM)
                # Uᵀ = (A·Q8)ᵀ + d_g'·Aᵀ
                nc.vector.scalar_tensor_tensor(
                    pa[:], aT[:, cb * R:(cb + 1) * R], d_cols[:, cb:cb + 1],
                    pa[:], op0=OP.mult, op1=OP.add)
                pent = pen_t_tile(cb)
                junk = workp.tile([P, TN], F16, tag="w1", name="junkU")
                nc.vector.scalar_tensor_tensor(
                    junk[:], pa[:], 1.0, pent[:], op0=OP.mult, op1=OP.mult,
                    accum_out=c3cols[:, cb:cb + 1])

            # ============ final reduction ==================================
            c2v = colp.tile([P, 1], F32, tag="c2v")
            c3v = colp.tile([P, 1], F32, tag="c3v")
            nc.vector.reduce_sum(c2v[:], c2cols[:], axis=AX.X)
            nc.vector.reduce_sum(c3v[:], c3cols[:], axis=AX.X)
            tot = colp.tile([P, 1], F32, tag="tot")
            nc.vector.tensor_scalar(tot[:], c2v[:], 0.5, None, OP.mult)
            nc.vector.tensor_tensor(tot[:], tot[:], c1v[:], OP.add)
            nc.vector.tensor_scalar(c3v[:], c3v[:], 1.0 / 3.0, None, OP.mult)
            nc.vector.tensor_tensor(tot[:], tot[:], c3v[:], OP.add)

            ones_c = constp.tile([P, 1], F32, tag="ones_c")
            nc.vector.memset(ones_c[:], 1.0)
            fin = ps_tile("pa0", shape=[1, 8])
            nc.tensor.matmul(fin[:, 0:1], tot[:], ones_c[:], start=True,
                             stop=True)
            lsb = colp.tile([1, 8], F32, tag="lsb")
            nc.vector.memset(lsb[:], 0.0)
            nc.scalar.activation(lsb[:, 0:1], fin[:, 0:1], ACT.Copy,
                                 scale=1.0 / float(B))
            nc.sync.dma_start(red_in[:, :], lsb[:])
            nc.gpsimd.collective_compute(
                "AllReduce", OP.add, replica_groups=rg,
                ins=[red_in.ap().opt()], outs=[red_out.ap().opt()])
            nc.sync.dma_start(loss_out[:, :], red_out[0:1, 0:1])

    nc.compile()
    return nc


def make_inputs(features, scores, labels, B, D, NC):
    """Build the per-core input maps from full inputs."""
    R = B // NC
    P = 128
    NMT = R // P
    NNT = B // R
    f = np.ascontiguousarray(features, dtype=np.float32)
    s = np.ascontiguousarray(scores, dtype=np.float32).reshape(B)
    lab = np.asarray(labels).astype(np.float32).reshape(B)
    ftf = np.ascontiguousarray(f.T)
    in_maps = []
    for c in range(NC):
        sh = slice(c * R, (c + 1) * R)
        onehot = np.zeros((1, NNT), dtype=np.float32)
        onehot[0, c] = 1.0
        in_maps.append({
            "ft2": np.ascontiguousarray(2.0 * f[sh].T).astype(ml_dtypes.bfloat16),
            "ftf": ftf.astype(ml_dtypes.bfloat16),
            "fsh": np.ascontiguousarray(f[sh]).astype(ml_dtypes.bfloat16),
            "srow": s.reshape(1, B),
            "srow16": s.reshape(1, B).astype(np.float16),
            "smyrow16": np.ascontiguousarray(s[sh]).reshape(1, R)
            .astype(np.float16),
            "scols": np.ascontiguousarray(s[sh].reshape(NMT, P).T),
            "lrow16": lab.reshape(1, B).astype(np.float16),
            "lcols": np.ascontiguousarray(lab[sh].reshape(NMT, P).T),
            "dsel": onehot,
        })
    return in_maps


_cached = {}


def kernel(features, scores, labels):
    B, D = features.shape
    NC = 8
    key = (B, D)
    if key not in _cached:
        _cached[key] = build_program(B=B, D=D, NC=NC)
    nc = _cached[key]
    from concourse.bass_utils import run_bass_kernel_spmd
    in_maps = make_inputs(features, scores, labels, B, D, NC)
    res = run_bass_kernel_spmd(nc, in_maps, core_ids=list(range(NC)))
    out = res.results[0]["loss"]
    return np.float32(out.reshape(())[()])


# revision 44
# speedup vs baseline: 1.1596x; 1.1596x over previous
"""Trainium2 Bass kernel for nn_DirectedODRLoss (retrieval_knn).

Math (B=4096, D=256, k=25, scales (1,2,3)):
    dist²(i,j) = |f_i|² + |f_j|² − 2 f_i·f_j ;  y := −dist²  (f32 strips;
        bf16 Gram GEMM; −|f_i|² per-partition scalar and −|f_j|² f32 row
        strip folded into one DVE scalar_tensor_tensor on the PSUM drain)
    topk: per row, candidates = top-8 of each of 16 256-wide chunks (16
        f32 max8 passes) → top-25 of the 128 candidates via max8 +
        match_replace;  τ_i := 25th largest y;  σ_i = mean(sqrt(−y+eps))
    mutual knn mask:  y symmetric ⇒ mutual(i,j) = [y_ij ≥ max(τ_i, τ_j)]
    drop = [max(τ_i,τ_j) > y] | [lab_j < lab_i];  e = y·rσ_j (fp16)
    Wn = exp(rσ_i·(e − 1024·drop))   (drop ⇒ exp(≤ −45/σ²) ⇒ 0 in fp16)
    S_i = ΣWn + 1,  P = (Wn + dsel·I)/S  (f32 in place; diag via a
        per-core one-hot dsel over column blocks), quantized to fp8 e4m3
    loss = (1/B)(C1 + C2/2 + C3/3) with
        C1 = <Wn/S, pen>,  C2 = <A, pen>,  C3 = <U, pen>,
        A = P²  (computed transposed: Aᵀ chunks, lhsT = raw pfull column
        strips, rhs = SBUF-resident P_shardᵀ),  U = A·P (Uᵀ likewise),
        pen_ij = relu(s_i − s_j) (fp16).

Sharding: rows split across 8 cores. P all-gathered in fp8; both B³ GEMMs
run in fp8 DoubleRow mode (0.5 cyc/row); Aᵀ/Uᵀ chunks consumed against
recomputed penᵀ tiles via fused scalar_tensor_tensor accumulations.
Final scalars all-reduced.
"""

import ml_dtypes
import numpy as np

import concourse.bacc as bacc
import concourse.bass as bass
import concourse.mybir as mybir
import concourse.tile as tile
from concourse.masks import make_identity

F32 = mybir.dt.float32
F16 = mybir.dt.float16
BF16 = mybir.dt.bfloat16
F8 = mybir.dt.float8e4
AX = mybir.AxisListType
OP = mybir.AluOpType
ACT = mybir.ActivationFunctionType
DRM = mybir.MatmulPerfMode.DoubleRow

EPS = 1e-8
KNN = 25
DROP_SHIFT = 1024.0
NEG_FILL = -1e30


def build_program(B=4096, D=256, NC=8):
    P = 128
    R = B // NC            # rows per core (512)
    NMT = R // P           # row tiles per core (4)
    TN = R                 # column tile (512)
    NNT = B // TN          # column tiles (8)
    KC = B // P            # 128-row chunks of B (32)
    KC2 = KC // 2          # DoubleRow steps (16)
    GK = D // P            # contraction chunks for the Gram GEMM (2)
    TC = 16                # topk candidate chunks per row
    CW = B // TC           # chunk width (256)

    nc = bacc.Bacc("TRN2", target_bir_lowering=False, debug=False,
                   num_devices=NC)

    # ---- I/O ----------------------------------------------------------------
    ft2 = nc.dram_tensor("ft2", [D, R], BF16, kind="ExternalInput")    # 2·F_shardᵀ
    ftf = nc.dram_tensor("ftf", [D, B], BF16, kind="ExternalInput")    # Fᵀ (full)
    fsh = nc.dram_tensor("fsh", [R, D], BF16, kind="ExternalInput")    # F shard
    srow = nc.dram_tensor("srow", [1, B], F32, kind="ExternalInput")   # scores
    srow16 = nc.dram_tensor("srow16", [1, B], F16, kind="ExternalInput")
    smyrow16 = nc.dram_tensor("smyrow16", [1, R], F16, kind="ExternalInput")
    scols = nc.dram_tensor("scols", [P, NMT], F32, kind="ExternalInput")
    lrow16 = nc.dram_tensor("lrow16", [1, B], F16, kind="ExternalInput")
    lcols = nc.dram_tensor("lcols", [P, NMT], F32, kind="ExternalInput")
    dsel = nc.dram_tensor("dsel", [1, NNT], F32, kind="ExternalInput")  # one-hot(rank)
    loss_out = nc.dram_tensor("loss", [1, 1], F32, kind="ExternalOutput")

    NMT0 = (B // NC) // 128
    # ---- internal DRAM ------------------------------------------------------
    # P is all-gathered in four row-quarters (one per mt tile) so the AGs
    # overlap stage W. Quarter q of pfull holds rank-major 128-row blocks:
    # global row c*R + q*128 + p ↔ pfull_q[c*128 + p].
    pn_half = [nc.dram_tensor(f"pn_half{h}", [P, B], F8) for h in range(NMT0)]
    pfull_h = [nc.dram_tensor(f"pfull{h}", [NC * P, B], F8,
                              addr_space="Shared") for h in range(NMT0)]
    stats_in = nc.dram_tensor("stats_in", [1, 2 * R], F32)
    stats_out = nc.dram_tensor("stats_out", [NC, 2 * R], F32, addr_space="Shared")
    d_dram = nc.dram_tensor("d_dram", [1, R], F32)
    d_out = nc.dram_tensor("d_out", [NC, R], F32, addr_space="Shared")
    red_in = nc.dram_tensor("red_in", [1, 8], F32)
    red_out = nc.dram_tensor("red_out", [1, 8], F32, addr_space="Shared")

    rg = [list(range(NC))]

    with tile.TileContext(nc) as tc:
        with (
            tc.tile_pool(name="const", bufs=1) as constp,
            tc.tile_pool(name="io", bufs=2) as iop,
            tc.tile_pool(name="pnp", bufs=1) as pnp,
            tc.tile_pool(name="big", bufs=1) as bigp,
            tc.tile_pool(name="strip", bufs=1) as stripp,
            tc.tile_pool(name="cols", bufs=1) as colp,
            tc.tile_pool(name="work", bufs=2) as workp,
            tc.tile_pool(name="psum", bufs=1, space="PSUM") as psump,
        ):
            def ps_tile(tag, shape=None, dtype=F32):
                return psump.tile(shape or [P, TN], dtype, tag=tag, name=tag)

            # Gram operands (bf16, cached in SBUF, reused per mt tile)
            ft2_sb = constp.tile([P, GK * R], BF16, tag="ft2_sb")
            for g in range(GK):
                nc.sync.dma_start(ft2_sb[:, g * R:(g + 1) * R],
                                  ft2[g * P:(g + 1) * P, :])
            ftf_sb = bigp.tile([P, GK * B], BF16, tag="fbuf", name="ftf_sb")
            for g in range(GK):
                nc.sync.dma_start(ftf_sb[:, g * B:(g + 1) * B],
                                  ftf[g * P:(g + 1) * P, :])

            # ============ stage A: squared norms ============================
            # full |f|²: square ftf (f32 out), column-reduce via ones-matmul
            # broadcast to all 128 partitions (lhsT = all-ones matrix)
            sq2 = bigp.tile([P, GK * B], F32, tag="ybuf", name="sq2")
            for g in range(GK):
                nc.scalar.activation(sq2[:, g * B:(g + 1) * B],
                                     ftf_sb[:, g * B:(g + 1) * B], ACT.Square)
            ones_m = constp.tile([P, P], F32, tag="ones_m")
            nc.vector.memset(ones_m[:], 1.0)
            sqj_b = stripp.tile([P, B], F32, tag="sqj_b")
            for ntb in range(NNT):
                psq = ps_tile(f"pa{ntb % 4}")
                for g in range(GK):
                    nc.tensor.matmul(
                        psq[:], ones_m[:],
                        sq2[:, g * B + ntb * TN: g * B + (ntb + 1) * TN],
                        start=(g == 0), stop=(g == GK - 1))
                nc.scalar.activation(sqj_b[:, ntb * TN:(ntb + 1) * TN],
                                     psq[:], ACT.Copy)
            # shard |f|² in column layout (per-partition scalars, negated)
            sqcs = colp.tile([P, NMT], F32, tag="sqcs")
            fshb = iop.tile([P, NMT * D], BF16, tag="fblk", name="fshb")
            nc.sync.dma_start(
                fshb[:].rearrange("p (t d) -> p t d", t=NMT),
                bass.AP(fsh, 0, [[D, P], [P * D, NMT], [1, D]]))
            for q in range(NMT):
                scr = workp.tile([P, D], F32, tag="sqscr")
                nc.scalar.activation(scr[:], fshb[:, q * D:(q + 1) * D],
                                     ACT.Square, accum_out=sqcs[:, q:q + 1])
            sqcsr = colp.tile([P, NMT], F32, tag="sqcsr")
            nc.vector.tensor_scalar(sqcsr[:], sqcs[:], -1.0, None, OP.mult)

            # misc broadcast loads
            dsel_c = colp.tile([P, NNT], F32, tag="dsel_c")
            nc.sync.dma_start(dsel_c[:], bass.AP(dsel, 0, [[0, P], [1, NNT]]))
            lab_c = colp.tile([P, NMT], F32, tag="lab_c")
            s_c = colp.tile([P, NMT], F32, tag="s_c")
            nc.sync.dma_start(lab_c[:], lcols[:, :])
            nc.sync.dma_start(s_c[:], scols[:, :])
            lab_b = stripp.tile([P, B], F16, tag="lab_b")
            s_b = stripp.tile([P, B], F16, tag="s_b")
            smy_b = constp.tile([P, R], F16, tag="smy_b")
            nc.sync.dma_start(lab_b[:], bass.AP(lrow16, 0, [[0, P], [1, B]]))
            nc.sync.dma_start(s_b[:], bass.AP(srow16, 0, [[0, P], [1, B]]))
            nc.sync.dma_start(smy_b[:], bass.AP(smyrow16, 0, [[0, P], [1, R]]))
            sfc = colp.tile([P, KC], F32, tag="sfc")       # −s_g, col layout
            nc.sync.dma_start(sfc[:], bass.AP(srow, 0, [[1, P], [P, KC]]))
            nc.vector.tensor_scalar(sfc[:], sfc[:], -1.0, None, OP.mult)

            ident16 = constp.tile([P, P], F16, tag="ident16")
            make_identity(nc, ident16[:])
            ndsel_c = colp.tile([P, NNT], F32, tag="ndsel_c")
            nc.vector.tensor_scalar(ndsel_c[:], dsel_c[:], -1.0, None, OP.mult)
            eps_c = constp.tile([P, 1], F32, tag="eps_c")
            nc.vector.memset(eps_c[:], EPS)

            # ============ stage B: Gram → y (f32) + chunked topk ============
            y_all = bigp.tile([P, NMT * B], F32, tag="ybuf", name="y_all")
            cands = colp.tile([P, TC * 8], F32, tag="cands")
            vals = colp.tile([P, 32], F32, tag="vals")
            yt_cols = colp.tile([P, NMT], F32, tag="yt_cols")
            rs_cols = colp.tile([P, NMT], F32, tag="rs_cols")
            ssum = colp.tile([P, NMT], F32, tag="ssum")
            for mt in range(NMT):
                gps = [ps_tile(f"pa{ntb}") for ntb in range(NNT)]
                for g in range(GK):
                    for ntb in range(NNT):
                        nc.tensor.matmul(
                            gps[ntb][:],
                            ft2_sb[:, g * R + mt * P: g * R + (mt + 1) * P],
                            ftf_sb[:, g * B + ntb * TN: g * B + (ntb + 1) * TN],
                            start=(g == 0), stop=(g == GK - 1))
                ys = y_all[:, mt * B:(mt + 1) * B]
                for ntb in range(NNT):
                    # y = (gram − |f_i|²) − |f_j|²   (f32 out)
                    nc.vector.scalar_tensor_tensor(
                        ys[:, ntb * TN:(ntb + 1) * TN], gps[ntb][:],
                        sqcsr[:, mt:mt + 1], sqj_b[:, ntb * TN:(ntb + 1) * TN],
                        op0=OP.add, op1=OP.subtract)
                # candidates: top-8 of each 256-wide chunk
                for t in range(TC):
                    nc.vector.max(out=cands[:, t * 8:(t + 1) * 8],
                                  in_=ys[:, t * CW:(t + 1) * CW])
                # top-25 of the 128 candidates
                ca = workp.tile([P, TC * 8], F32, tag="ca", name="ca")
                cb = workp.tile([P, TC * 8], F32, tag="cb", name="cb")
                nc.vector.max(out=vals[:, 0:8], in_=cands[:])
                nc.vector.match_replace(out=ca[:], in_to_replace=vals[:, 0:8],
                                        in_values=cands[:], imm_value=NEG_FILL)
                nc.vector.max(out=vals[:, 8:16], in_=ca[:])
                nc.vector.match_replace(out=cb[:], in_to_replace=vals[:, 8:16],
                                        in_values=ca[:], imm_value=NEG_FILL)
                nc.vector.max(out=vals[:, 16:24], in_=cb[:])
                nc.vector.match_replace(out=ca[:], in_to_replace=vals[:, 16:24],
                                        in_values=cb[:], imm_value=NEG_FILL)
                nc.vector.max(out=vals[:, 24:32], in_=ca[:])
                # τ_i = 25th largest y
                nc.vector.tensor_copy(yt_cols[:, mt:mt + 1], vals[:, 24:25])
                # σ_i = mean sqrt(max(d,0)+eps) over 25 NN;  d = −y
                c25 = workp.tile([P, KNN], F32, tag="c25")
                nc.vector.tensor_scalar(c25[:], vals[:, 0:KNN], 0.0, None,
                                        OP.min)
                s25 = workp.tile([P, KNN], F32, tag="s25")
                nc.scalar.activation(s25[:], c25[:], ACT.Sqrt,
                                     bias=eps_c[:, 0:1], scale=-1.0,
                                     accum_out=ssum[:, mt:mt + 1])
            nc.vector.reciprocal(rs_cols[:], ssum[:])
            nc.vector.tensor_scalar(rs_cols[:], rs_cols[:], float(KNN), None,
                                    OP.mult)

            # stats all-gather: flat per-rank [τ(R) ++ rσ(R)] (f32), both in
            # shard-row order g_local = c*128 + p.
            nc.sync.dma_start(bass.AP(stats_in, 0, [[1, P], [P, NMT]]),
                              yt_cols[:])
            nc.sync.dma_start(bass.AP(stats_in, R, [[1, P], [P, NMT]]),
                              rs_cols[:])
            nc.gpsimd.collective_compute(
                "AllGather", OP.bypass, replica_groups=rg,
                ins=[stats_in.ap().opt()], outs=[stats_out.ap().opt()])

            def stat_bcast_ap(off):
                return bass.AP(stats_out, off, [[0, P], [2 * R, NC], [1, R]])

            yt_b = stripp.tile([P, B], F32, tag="yt_b")
            nc.sync.dma_start(yt_b[:].rearrange("a (r q) -> a r q", r=NC),
                              stat_bcast_ap(0))
            # rσ strip: stage f32 into sqj_b's buffer (dead after stage B),
            # convert to fp16
            rs_f32 = stripp.tile([P, B], F32, tag="sqj_b", name="rs_f32")
            nc.sync.dma_start(rs_f32[:].rearrange("a (r q) -> a r q", r=NC),
                              stat_bcast_ap(R))
            rs_b = stripp.tile([P, B], F16, tag="rs_b")
            nc.vector.tensor_copy(rs_b[:], rs_f32[:])

            # ============ stage W: Wn, S, P, C1 =============================
            srcols = colp.tile([P, NMT * NNT], F32, tag="srcols")
            c1cols = colp.tile([P, NMT * NNT], F32, tag="c1cols")
            invS = colp.tile([P, NMT], F32, tag="invS")
            Scol = colp.tile([P, NMT], F32, tag="Scol")
            pn8 = [pnp.tile([P, B], F8, tag="pn8", name=f"pn8_{mt}")
                   for mt in range(NMT)]
            for mt in range(NMT):
                ys = y_all[:, mt * B:(mt + 1) * B]
                for ntb in range(NNT):
                    yt_t = ys[:, ntb * TN:(ntb + 1) * TN]
                    # drop = [max(τ_j, τ_i) > y] | [lab_j < lab_i]
                    drop1 = workp.tile([P, TN], F16, tag="w1", name="drop1")
                    nc.vector.scalar_tensor_tensor(
                        drop1[:], yt_b[:, ntb * TN:(ntb + 1) * TN],
                        yt_cols[:, mt:mt + 1], yt_t,
                        op0=OP.max, op1=OP.is_gt)
                    drop = workp.tile([P, TN], F16, tag="w2", name="drop")
                    nc.vector.scalar_tensor_tensor(
                        drop[:], lab_b[:, ntb * TN:(ntb + 1) * TN],
                        lab_c[:, mt:mt + 1], drop1[:],
                        op0=OP.is_lt, op1=OP.max)
                    # e = y·rσ_j (fp16); e2 = e − 1024·drop
                    e = workp.tile([P, TN], F16, tag="w3", name="e")
                    nc.vector.tensor_tensor(
                        e[:], yt_t, rs_b[:, ntb * TN:(ntb + 1) * TN], OP.mult)
                    e2 = workp.tile([P, TN], F16, tag="w1", name="e2")
                    nc.vector.scalar_tensor_tensor(
                        e2[:], drop[:], -DROP_SHIFT, e[:],
                        op0=OP.mult, op1=OP.add)
                    # Wn = exp(rσ_i·e2), written over the f32 y strip,
                    # accum → ΣWn
                    nc.scalar.activation(yt_t, e2[:], ACT.Exp,
                                         scale=rs_cols[:, mt:mt + 1],
                                         accum_out=srcols[:, mt * NNT + ntb:
                                                          mt * NNT + ntb + 1])

                # S = ΣWn + 1 ;  invS = 1/S
                nc.vector.reduce_sum(Scol[:, mt:mt + 1],
                                     srcols[:, mt * NNT:(mt + 1) * NNT],
                                     axis=AX.X)
                nc.vector.tensor_scalar(Scol[:, mt:mt + 1], Scol[:, mt:mt + 1],
                                        1.0, None, OP.add)
                nc.vector.reciprocal(invS[:, mt:mt + 1], Scol[:, mt:mt + 1])
                # Q = offdiag(P) = (Wn − dsel·I)/S  (diagonal handled exactly
                # via d = 2/S in the GEMM correction terms)
                for ntb in range(NNT):
                    dslice = ys[:, ntb * TN + mt * P: ntb * TN + (mt + 1) * P]
                    nc.vector.scalar_tensor_tensor(
                        dslice, ident16[:], ndsel_c[:, ntb:ntb + 1], dslice,
                        op0=OP.mult, op1=OP.add)
                    nc.vector.tensor_scalar(ys[:, ntb * TN:(ntb + 1) * TN],
                                            ys[:, ntb * TN:(ntb + 1) * TN],
                                            invS[:, mt:mt + 1], None, OP.mult)
                    # C1 partial: Σ Q·pen = Σ Wn·pen/S  (pen_ii = 0)
                    pen = workp.tile([P, TN], F16, tag="w3", name="pen")
                    nc.scalar.activation(pen[:], s_b[:, ntb * TN:(ntb + 1) * TN],
                                         ACT.Relu, bias=s_c[:, mt:mt + 1],
                                         scale=-1.0)
                    junk = workp.tile([P, TN], F16, tag="w1", name="junk")
                    nc.vector.scalar_tensor_tensor(
                        junk[:], ys[:, ntb * TN:(ntb + 1) * TN], 1.0, pen[:],
                        op0=OP.mult, op1=OP.mult,
                        accum_out=c1cols[:, mt * NNT + ntb:
                                         mt * NNT + ntb + 1])
                nc.scalar.activation(pn8[mt][:], ys, ACT.Copy)
                nc.sync.dma_start(
                    bass.AP(pn_half[mt], 0, [[B, P], [1, B]]), pn8[mt][:])
                nc.gpsimd.collective_compute(
                    "AllGather", OP.bypass, replica_groups=rg,
                    ins=[pn_half[mt].ap().opt()],
                    outs=[pfull_h[mt].ap().opt()])

            # C1 finalize (1/S already folded in via the Q strip)
            c1v = colp.tile([P, 1], F32, tag="c1v")
            nc.vector.reduce_sum(c1v[:], c1cols[:], axis=AX.X)

            # ============ all-gather of diag d = 2/S (f32) ==================
            dloc = colp.tile([P, NMT], F32, tag="dloc")
            nc.vector.tensor_scalar(dloc[:], invS[:], 2.0, None, OP.mult)
            nc.sync.dma_start(bass.AP(d_dram, 0, [[1, P], [P, NMT]]), dloc[:])
            nc.gpsimd.collective_compute(
                "AllGather", OP.bypass, replica_groups=rg,
                ins=[d_dram.ap().opt()], outs=[d_out.ap().opt()])
            # d in column layout over full B, d² gated by the diag one-hot,
            # and shard d broadcast along the free axis
            d_cols = colp.tile([P, KC], F32, tag="d_cols")
            nc.sync.dma_start(d_cols[:], bass.AP(d_out, 0, [[1, P], [P, KC]]))
            d2g = colp.tile([P, KC], F32, tag="d2g")
            nc.vector.tensor_tensor(d2g[:], d_cols[:], d_cols[:], OP.mult)
            for ntb in range(NNT):
                nc.vector.tensor_scalar(
                    d2g[:, ntb * NMT:(ntb + 1) * NMT],
                    d2g[:, ntb * NMT:(ntb + 1) * NMT],
                    dsel_c[:, ntb:ntb + 1], None, OP.mult)
            d_strip = constp.tile([P, R], F32, tag="d_strip")
            nc.sync.dma_start(d_strip[:], bass.AP(d_dram, 0, [[0, P], [1, R]]))

            # ============ P_shardᵀ (fp8, SBUF) via PE transposes ============
            # transpose the f32 P strips (2 cyc/row), cast fp8 on psum drain
            idf32 = constp.tile([P, P], F32, tag="idf32")
            make_identity(nc, idf32[:])
            psT = bigp.tile([P, KC * R], F8, tag="ptbuf")
            for ntb in range(NNT):
                for u in range(4):
                    ptb = ps_tile(f"pa{(ntb * 4 + u) % 4}", dtype=F32)
                    for mt in range(NMT):
                        nc.tensor.transpose(
                            ptb[:, mt * P:(mt + 1) * P],
                            y_all[:, mt * B + ntb * TN + u * P:
                                  mt * B + ntb * TN + (u + 1) * P],
                            idf32[:])
                    nc.scalar.activation(
                        psT[:, (ntb * 4 + u) * R:(ntb * 4 + u + 1) * R],
                        ptb[:], ACT.Copy)

            # ============ GEMM1: Aᵀ chunks = (P²)ᵀ, consume C2, cast fp8 ====
            c2cols = colp.tile([P, KC], F32, tag="c2cols")
            c3cols = colp.tile([P, KC], F32, tag="c3cols")
            aT = bigp.tile([P, KC * R], F8, tag="atbuf")

            def lhs_strip_load(cbi, phase):
                # ls[p, j, m] with global row-block j = c*4 + q; half h=q//2
                # holds j ≡ 2h, 2h+1 (mod 4) at rows c*256 + (q%2)*128 + p.
                ls = iop.tile([P, KC * P], F8, tag="lhs_strip",
                              name=f"ls{phase}_{cbi}")
                ls4 = ls[:].rearrange("p (c u m) -> p c u m", c=NC, u=4)
                for h in range(4):
                    nc.sync.dma_start(
                        ls4[:, :, h, :],
                        bass.AP(pfull_h[h], cbi * P,
                                [[B, P], [P * B, NC], [1, P]]))
                return ls

            def pen_t_tile(cbi):
                # penᵀ chunk [g-block, i]: relu(s_i − s_g)
                pt = workp.tile([P, TN], F16, tag="w3", name=f"pent{cbi}")
                nc.scalar.activation(pt[:], smy_b[:], ACT.Relu,
                                     bias=sfc[:, cbi:cbi + 1], scale=1.0)
                return pt

            for cb in range(KC):
                ls = lhs_strip_load(cb, 0)
                pa = ps_tile(f"pa{cb % 4}")
                for kb in range(KC2):
                    nc.tensor.matmul(
                        pa[:],
                        ls[:, 2 * kb * P:(2 * kb + 2) * P]
                        .rearrange("p (k m) -> p k m", k=2),
                        psT[:, 2 * kb * R:(2 * kb + 2) * R]
                        .rearrange("p (k n) -> p k n", k=2),
                        start=(kb == 0), stop=(kb == KC2 - 1), perf_mode=DRM)
                # diagonal corrections:
                # Aᵀ = (Q8²)ᵀ + d_g'·Q8ᵀ + Q8ᵀ·d_i + dsel·diag(d²)
                qt = psT[:, cb * R:(cb + 1) * R]
                nc.vector.scalar_tensor_tensor(
                    pa[:], qt, d_cols[:, cb:cb + 1], pa[:],
                    op0=OP.mult, op1=OP.add)
                tdi = workp.tile([P, TN], F32, tag="sqscr", name="tdi")
                nc.vector.tensor_tensor(tdi[:], qt, d_strip[:], OP.mult)
                nc.vector.tensor_tensor(pa[:], pa[:], tdi[:], OP.add)
                dgo = (cb % NMT) * P
                nc.vector.scalar_tensor_tensor(
                    pa[:, dgo:dgo + P], ident16[:], d2g[:, cb:cb + 1],
                    pa[:, dgo:dgo + P], op0=OP.mult, op1=OP.add)
                pent = pen_t_tile(cb)
                junk = workp.tile([P, TN], F16, tag="w1", name="junkA")
                nc.vector.scalar_tensor_tensor(
                    junk[:], pa[:], 1.0, pent[:], op0=OP.mult, op1=OP.mult,
                    accum_out=c2cols[:, cb:cb + 1])
                nc.scalar.activation(aT[:, cb * R:(cb + 1) * R], pa[:],
                                     ACT.Copy)

            # ============ GEMM2: Uᵀ chunks = (A·P)ᵀ, consume C3 =============
            for cb in range(KC):
                ls = lhs_strip_load(cb, 1)
                pa = ps_tile(f"pa{4 + cb % 4}")
                for kb in range(KC2):
                    nc.tensor.matmul(
                        pa[:],
                        ls[:, 2 * kb * P:(2 * kb + 2) * P]
                        .rearrange("p (k m) -> p k m", k=2),
                        aT[:, 2 * kb * R:(2 * kb + 2) * R]
                        .rearrange("p (k n) -> p k n", k=2),
                        start=(kb == 0), stop=(kb == KC2 - 1), perf_mode=DRM)
                # Uᵀ = (A·Q8)ᵀ + d_g'·Aᵀ
                nc.vector.scalar_tensor_tensor(
                    pa[:], aT[:, cb * R:(cb + 1) * R], d_cols[:, cb:cb + 1],
                    pa[:], op0=OP.mult, op1=OP.add)
                pent = pen_t_tile(cb)
                junk = workp.tile([P, TN], F16, tag="w1", name="junkU")
                nc.vector.scalar_tensor_tensor(
                    junk[:], pa[:], 1.0, pent[:], op0=OP.mult, op1=OP.mult,
                    accum_out=c3cols[:, cb:cb + 1])

            # ============ final reduction ==================================
            c2v = colp.tile([P, 1], F32, tag="c2v")
            c3v = colp.tile([P, 1], F32, tag="c3v")
            nc.vector.reduce_sum(c2v[:], c2cols[:], axis=AX.X)
            nc.vector.reduce_sum(c3v[:], c3cols[:], axis=AX.X)
            tot = colp.tile([P, 1], F32, tag="tot")
            nc.vector.tensor_scalar(tot[:], c2v[:], 0.5, None, OP.mult)
            nc.vector.tensor_tensor(tot[:], tot[:], c1v[:], OP.add)
            nc.vector.tensor_scalar(c3v[:], c3v[:], 1.0 / 3.0, None, OP.mult)
            nc.vector.tensor_tensor(tot[:], tot[:], c3v[:], OP.add)

            ones_c = constp.tile([P, 1], F32, tag="ones_c")
            nc.vector.memset(ones_c[:], 1.0)
            fin = ps_tile("pa0", shape=[1, 8])
            nc.tensor.matmul(fin[:, 0:1], tot[:], ones_c[:], start=True,
                             stop=True)
            lsb = colp.tile([1, 8], F32, tag="lsb")
            nc.vector.memset(lsb[:], 0.0)
            nc.scalar.activation(lsb[:, 0:1], fin[:, 0:1], ACT.Copy,
                                 scale=1.0 / float(B))
            nc.sync.dma_start(red_in[:, :], lsb[:])
            nc.gpsimd.collective_compute(
                "AllReduce", OP.add, replica_groups=rg,
                ins=[red_in.ap().opt()], outs=[red_out.ap().opt()])
            nc.sync.dma_start(loss_out[:, :], red_out[0:1, 0:1])

    nc.compile()
    return nc


def make_inputs(features, scores, labels, B, D, NC):
    """Build the per-core input maps from full inputs."""
    R = B // NC
    P = 128
    NMT = R // P
    NNT = B // R
    f = np.ascontiguousarray(features, dtype=np.float32)
    s = np.ascontiguousarray(scores, dtype=np.float32).reshape(B)
    lab = np.asarray(labels).astype(np.float32).reshape(B)
    ftf = np.ascontiguousarray(f.T)
    in_maps = []
    for c in range(NC):
        sh = slice(c * R, (c + 1) * R)
        onehot = np.zeros((1, NNT), dtype=np.float32)
        onehot[0, c] = 1.0
        in_maps.append({
            "ft2": np.ascontiguousarray(2.0 * f[sh].T).astype(ml_dtypes.bfloat16),
            "ftf": ftf.astype(ml_dtypes.bfloat16),
            "fsh": np.ascontiguousarray(f[sh]).astype(ml_dtypes.bfloat16),
            "srow": s.reshape(1, B),
            "srow16": s.reshape(1, B).astype(np.float16),
            "smyrow16": np.ascontiguousarray(s[sh]).reshape(1, R)
            .astype(np.float16),
            "scols": np.ascontiguousarray(s[sh].reshape(NMT, P).T),
            "lrow16": lab.reshape(1, B).astype(np.float16),
            "lcols": np.ascontiguousarray(lab[sh].reshape(NMT, P).T),
            "dsel": onehot,
        })
    return in_maps


_cached = {}


def kernel(features, scores, labels):
    B, D = features.shape
    NC = 8
    key = (B, D)
    if key not in _cached:
        _cached[key] = build_program(B=B, D=D, NC=NC)
    nc = _cached[key]
    from concourse.bass_utils import run_bass_kernel_spmd
    in_maps = make_inputs(features, scores, labels, B, D, NC)
    res = run_bass_kernel_spmd(nc, in_maps, core_ids=list(range(NC)))
    out = res.results[0]["loss"]
    return np.float32(out.reshape(())[()])


# revision 45
# speedup vs baseline: 1.1701x; 1.0091x over previous
"""Trainium2 Bass kernel for nn_DirectedODRLoss (retrieval_knn).

Math (B=4096, D=256, k=25, scales (1,2,3)):
    dist²(i,j) = |f_i|² + |f_j|² − 2 f_i·f_j ;  y := −dist²  (f32 strips;
        bf16 Gram GEMM; −|f_i|² per-partition scalar and −|f_j|² f32 row
        strip folded into one DVE scalar_tensor_tensor on the PSUM drain)
    topk: per row, candidates = top-8 of each of 16 256-wide chunks (16
        f32 max8 passes) → top-25 of the 128 candidates via max8 +
        match_replace;  τ_i := 25th largest y;  σ_i = mean(sqrt(−y+eps))
    mutual knn mask:  y symmetric ⇒ mutual(i,j) = [y_ij ≥ max(τ_i, τ_j)]
    drop = [max(τ_i,τ_j) > y] | [lab_j < lab_i];  e = y·rσ_j (fp16)
    Wn = exp(rσ_i·(e − 1024·drop))   (drop ⇒ exp(≤ −45/σ²) ⇒ 0 in fp16)
    S_i = ΣWn + 1,  P = (Wn + dsel·I)/S  (f32 in place; diag via a
        per-core one-hot dsel over column blocks), quantized to fp8 e4m3
    loss = (1/B)(C1 + C2/2 + C3/3) with
        C1 = <Wn/S, pen>,  C2 = <A, pen>,  C3 = <U, pen>,
        A = P²  (computed transposed: Aᵀ chunks, lhsT = raw pfull column
        strips, rhs = SBUF-resident P_shardᵀ),  U = A·P (Uᵀ likewise),
        pen_ij = relu(s_i − s_j) (fp16).

Sharding: rows split across 8 cores. P all-gathered in fp8; both B³ GEMMs
run in fp8 DoubleRow mode (0.5 cyc/row); Aᵀ/Uᵀ chunks consumed against
recomputed penᵀ tiles via fused scalar_tensor_tensor accumulations.
Final scalars all-reduced.
"""

import ml_dtypes
import numpy as np

import concourse.bacc as bacc
import concourse.bass as bass
import concourse.mybir as mybir
import concourse.tile as tile
from concourse.masks import make_identity

F32 = mybir.dt.float32
F16 = mybir.dt.float16
BF16 = mybir.dt.bfloat16
F8 = mybir.dt.float8e4
AX = mybir.AxisListType
OP = mybir.AluOpType
ACT = mybir.ActivationFunctionType
DRM = mybir.MatmulPerfMode.DoubleRow

EPS = 1e-8
KNN = 25
DROP_SHIFT = 1024.0
NEG_FILL = -1e30


def build_program(B=4096, D=256, NC=8):
    P = 128
    R = B // NC            # rows per core (512)
    NMT = R // P           # row tiles per core (4)
    TN = R                 # column tile (512)
    NNT = B // TN          # column tiles (8)
    KC = B // P            # 128-row chunks of B (32)
    KC2 = KC // 2          # DoubleRow steps (16)
    GK = D // P            # contraction chunks for the Gram GEMM (2)
    TC = 16                # topk candidate chunks per row
    CW = B // TC           # chunk width (256)

    nc = bacc.Bacc("TRN2", target_bir_lowering=False, debug=False,
                   num_devices=NC)

    # ---- I/O ----------------------------------------------------------------
    ft2 = nc.dram_tensor("ft2", [D, R], BF16, kind="ExternalInput")    # 2·F_shardᵀ
    ftf = nc.dram_tensor("ftf", [D, B], BF16, kind="ExternalInput")    # Fᵀ (full)
    fsh = nc.dram_tensor("fsh", [R, D], BF16, kind="ExternalInput")    # F shard
    srow = nc.dram_tensor("srow", [1, B], F32, kind="ExternalInput")   # scores
    srow16 = nc.dram_tensor("srow16", [1, B], F16, kind="ExternalInput")
    smyrow16 = nc.dram_tensor("smyrow16", [1, R], F16, kind="ExternalInput")
    scols = nc.dram_tensor("scols", [P, NMT], F32, kind="ExternalInput")
    lrow16 = nc.dram_tensor("lrow16", [1, B], F16, kind="ExternalInput")
    lcols = nc.dram_tensor("lcols", [P, NMT], F32, kind="ExternalInput")
    dsel = nc.dram_tensor("dsel", [1, NNT], F32, kind="ExternalInput")  # one-hot(rank)
    loss_out = nc.dram_tensor("loss", [1, 1], F32, kind="ExternalOutput")

    NMT0 = (B // NC) // 128
    # ---- internal DRAM ------------------------------------------------------
    # P is all-gathered in four row-quarters (one per mt tile) so the AGs
    # overlap stage W. Quarter q of pfull holds rank-major 128-row blocks:
    # global row c*R + q*128 + p ↔ pfull_q[c*128 + p].
    pn_half = [nc.dram_tensor(f"pn_half{h}", [P, B], F8) for h in range(NMT0)]
    pfull_h = [nc.dram_tensor(f"pfull{h}", [NC * P, B], F8,
                              addr_space="Shared") for h in range(NMT0)]
    stats_in = nc.dram_tensor("stats_in", [1, 2 * R], F32)
    stats_out = nc.dram_tensor("stats_out", [NC, 2 * R], F32, addr_space="Shared")
    d_dram = nc.dram_tensor("d_dram", [1, R], F32)
    d_out = nc.dram_tensor("d_out", [NC, R], F32, addr_space="Shared")
    red_in = nc.dram_tensor("red_in", [1, 8], F32)
    red_out = nc.dram_tensor("red_out", [1, 8], F32, addr_space="Shared")

    rg = [list(range(NC))]

    with tile.TileContext(nc) as tc:
        with (
            tc.tile_pool(name="const", bufs=1) as constp,
            tc.tile_pool(name="io", bufs=2) as iop,
            tc.tile_pool(name="pnp", bufs=1) as pnp,
            tc.tile_pool(name="big", bufs=1) as bigp,
            tc.tile_pool(name="strip", bufs=1) as stripp,
            tc.tile_pool(name="cols", bufs=1) as colp,
            tc.tile_pool(name="work", bufs=2) as workp,
            tc.tile_pool(name="psum", bufs=1, space="PSUM") as psump,
        ):
            def ps_tile(tag, shape=None, dtype=F32):
                return psump.tile(shape or [P, TN], dtype, tag=tag, name=tag)

            # Gram operands (bf16, cached in SBUF, reused per mt tile)
            ft2_sb = constp.tile([P, GK * R], BF16, tag="ft2_sb")
            for g in range(GK):
                nc.sync.dma_start(ft2_sb[:, g * R:(g + 1) * R],
                                  ft2[g * P:(g + 1) * P, :])
            ftf_sb = bigp.tile([P, GK * B], BF16, tag="fbuf", name="ftf_sb")
            for g in range(GK):
                nc.sync.dma_start(ftf_sb[:, g * B:(g + 1) * B],
                                  ftf[g * P:(g + 1) * P, :])

            # ============ stage A: squared norms ============================
            # full |f|²: square ftf (f32 out), column-reduce via ones-matmul
            # broadcast to all 128 partitions (lhsT = all-ones matrix)
            sq2 = bigp.tile([P, GK * B], F32, tag="ybuf", name="sq2")
            for g in range(GK):
                nc.scalar.activation(sq2[:, g * B:(g + 1) * B],
                                     ftf_sb[:, g * B:(g + 1) * B], ACT.Square)
            ones_m = constp.tile([P, P], F32, tag="ones_m")
            nc.vector.memset(ones_m[:], 1.0)
            sqj_b = stripp.tile([P, B], F32, tag="sqj_b")
            for ntb in range(NNT):
                psq = ps_tile(f"pa{ntb % 4}")
                for g in range(GK):
                    nc.tensor.matmul(
                        psq[:], ones_m[:],
                        sq2[:, g * B + ntb * TN: g * B + (ntb + 1) * TN],
                        start=(g == 0), stop=(g == GK - 1))
                nc.scalar.activation(sqj_b[:, ntb * TN:(ntb + 1) * TN],
                                     psq[:], ACT.Copy)
            # shard |f|² in column layout (per-partition scalars, negated)
            sqcs = colp.tile([P, NMT], F32, tag="sqcs")
            fshb = iop.tile([P, NMT * D], BF16, tag="fblk", name="fshb")
            nc.sync.dma_start(
                fshb[:].rearrange("p (t d) -> p t d", t=NMT),
                bass.AP(fsh, 0, [[D, P], [P * D, NMT], [1, D]]))
            for q in range(NMT):
                scr = workp.tile([P, D], F32, tag="sqscr")
                nc.scalar.activation(scr[:], fshb[:, q * D:(q + 1) * D],
                                     ACT.Square, accum_out=sqcs[:, q:q + 1])
            sqcsr = colp.tile([P, NMT], F32, tag="sqcsr")
            nc.vector.tensor_scalar(sqcsr[:], sqcs[:], -1.0, None, OP.mult)

            # misc broadcast loads
            dsel_c = colp.tile([P, NNT], F32, tag="dsel_c")
            nc.sync.dma_start(dsel_c[:], bass.AP(dsel, 0, [[0, P], [1, NNT]]))
            lab_c = colp.tile([P, NMT], F32, tag="lab_c")
            s_c = colp.tile([P, NMT], F32, tag="s_c")
            nc.sync.dma_start(lab_c[:], lcols[:, :])
            nc.sync.dma_start(s_c[:], scols[:, :])
            lab_b = stripp.tile([P, B], F16, tag="lab_b")
            s_b = stripp.tile([P, B], F16, tag="s_b")
            smy_b = constp.tile([P, R], F16, tag="smy_b")
            nc.sync.dma_start(lab_b[:], bass.AP(lrow16, 0, [[0, P], [1, B]]))
            nc.sync.dma_start(s_b[:], bass.AP(srow16, 0, [[0, P], [1, B]]))
            nc.sync.dma_start(smy_b[:], bass.AP(smyrow16, 0, [[0, P], [1, R]]))
            sfc = colp.tile([P, KC], F32, tag="sfc")       # −s_g, col layout
            nc.sync.dma_start(sfc[:], bass.AP(srow, 0, [[1, P], [P, KC]]))
            nc.vector.tensor_scalar(sfc[:], sfc[:], -1.0, None, OP.mult)

            ident16 = constp.tile([P, P], F16, tag="ident16")
            make_identity(nc, ident16[:])
            ndsel_c = colp.tile([P, NNT], F32, tag="ndsel_c")
            nc.vector.tensor_scalar(ndsel_c[:], dsel_c[:], -1.0, None, OP.mult)
            eps_c = constp.tile([P, 1], F32, tag="eps_c")
            nc.vector.memset(eps_c[:], EPS)

            # ============ stage B: Gram → y (f32) + chunked topk ============
            y_all = bigp.tile([P, NMT * B], F32, tag="ybuf", name="y_all")
            cands = colp.tile([P, TC * 8], F32, tag="cands")
            vals = colp.tile([P, 32], F32, tag="vals")
            yt_cols = colp.tile([P, NMT], F32, tag="yt_cols")
            rs_cols = colp.tile([P, NMT], F32, tag="rs_cols")
            ssum = colp.tile([P, NMT], F32, tag="ssum")
            for mt in range(NMT):
                gps = [ps_tile(f"pa{ntb}") for ntb in range(NNT)]
                for g in range(GK):
                    for ntb in range(NNT):
                        nc.tensor.matmul(
                            gps[ntb][:],
                            ft2_sb[:, g * R + mt * P: g * R + (mt + 1) * P],
                            ftf_sb[:, g * B + ntb * TN: g * B + (ntb + 1) * TN],
                            start=(g == 0), stop=(g == GK - 1))
                ys = y_all[:, mt * B:(mt + 1) * B]
                for ntb in range(NNT):
                    # y = (gram − |f_i|²) − |f_j|²   (f32 out)
                    nc.vector.scalar_tensor_tensor(
                        ys[:, ntb * TN:(ntb + 1) * TN], gps[ntb][:],
                        sqcsr[:, mt:mt + 1], sqj_b[:, ntb * TN:(ntb + 1) * TN],
                        op0=OP.add, op1=OP.subtract)
                # candidates: top-8 of each 256-wide chunk
                for t in range(TC):
                    nc.vector.max(out=cands[:, t * 8:(t + 1) * 8],
                                  in_=ys[:, t * CW:(t + 1) * CW])
                # top-25 of the 128 candidates
                ca = workp.tile([P, TC * 8], F32, tag="ca", name="ca")
                cb = workp.tile([P, TC * 8], F32, tag="cb", name="cb")
                nc.vector.max(out=vals[:, 0:8], in_=cands[:])
                nc.vector.match_replace(out=ca[:], in_to_replace=vals[:, 0:8],
                                        in_values=cands[:], imm_value=NEG_FILL)
                nc.vector.max(out=vals[:, 8:16], in_=ca[:])
                nc.vector.match_replace(out=cb[:], in_to_replace=vals[:, 8:16],
                                        in_values=ca[:], imm_value=NEG_FILL)
                nc.vector.max(out=vals[:, 16:24], in_=cb[:])
                nc.vector.match_replace(out=ca[:], in_to_replace=vals[:, 16:24],
                                        in_values=cb[:], imm_value=NEG_FILL)
                nc.vector.max(out=vals[:, 24:32], in_=ca[:])
                # τ_i = 25th largest y
                nc.vector.tensor_copy(yt_cols[:, mt:mt + 1], vals[:, 24:25])
                # σ_i = mean sqrt(max(d,0)+eps) over 25 NN;  d = −y
                c25 = workp.tile([P, KNN], F32, tag="c25")
                nc.vector.tensor_scalar(c25[:], vals[:, 0:KNN], 0.0, None,
                                        OP.min)
                s25 = workp.tile([P, KNN], F32, tag="s25")
                nc.scalar.activation(s25[:], c25[:], ACT.Sqrt,
                                     bias=eps_c[:, 0:1], scale=-1.0,
                                     accum_out=ssum[:, mt:mt + 1])
            nc.vector.reciprocal(rs_cols[:], ssum[:])
            nc.vector.tensor_scalar(rs_cols[:], rs_cols[:], float(KNN), None,
                                    OP.mult)

            # stats all-gather: flat per-rank [τ(R) ++ rσ(R)] (f32), both in
            # shard-row order g_local = c*128 + p.
            nc.sync.dma_start(bass.AP(stats_in, 0, [[1, P], [P, NMT]]),
                              yt_cols[:])
            nc.sync.dma_start(bass.AP(stats_in, R, [[1, P], [P, NMT]]),
                              rs_cols[:])
            nc.gpsimd.collective_compute(
                "AllGather", OP.bypass, replica_groups=rg,
                ins=[stats_in.ap().opt()], outs=[stats_out.ap().opt()])

            def stat_bcast_ap(off):
                return bass.AP(stats_out, off, [[0, P], [2 * R, NC], [1, R]])

            yt_b = stripp.tile([P, B], F32, tag="yt_b")
            nc.sync.dma_start(yt_b[:].rearrange("a (r q) -> a r q", r=NC),
                              stat_bcast_ap(0))
            # rσ strip: stage f32 into sqj_b's buffer (dead after stage B),
            # convert to fp16
            rs_f32 = stripp.tile([P, B], F32, tag="sqj_b", name="rs_f32")
            nc.sync.dma_start(rs_f32[:].rearrange("a (r q) -> a r q", r=NC),
                              stat_bcast_ap(R))
            rs_b = stripp.tile([P, B], F16, tag="rs_b")
            nc.vector.tensor_copy(rs_b[:], rs_f32[:])

            # ============ stage W: Wn, S, P, C1 =============================
            srcols = colp.tile([P, NMT * NNT], F32, tag="srcols")
            c1cols = colp.tile([P, NMT * NNT], F32, tag="c1cols")
            invS = colp.tile([P, NMT], F32, tag="invS")
            Scol = colp.tile([P, NMT], F32, tag="Scol")
            pn8 = [pnp.tile([P, B], F8, tag="pn8", name=f"pn8_{mt}")
                   for mt in range(NMT)]
            for mt in range(NMT):
                ys = y_all[:, mt * B:(mt + 1) * B]
                for ntb in range(NNT):
                    yt_t = ys[:, ntb * TN:(ntb + 1) * TN]
                    # drop = [max(τ_j, τ_i) > y] | [lab_j < lab_i]
                    drop1 = workp.tile([P, TN], F16, tag="w1", name="drop1")
                    nc.vector.scalar_tensor_tensor(
                        drop1[:], yt_b[:, ntb * TN:(ntb + 1) * TN],
                        yt_cols[:, mt:mt + 1], yt_t,
                        op0=OP.max, op1=OP.is_gt)
                    drop = workp.tile([P, TN], F16, tag="w2", name="drop")
                    nc.vector.scalar_tensor_tensor(
                        drop[:], lab_b[:, ntb * TN:(ntb + 1) * TN],
                        lab_c[:, mt:mt + 1], drop1[:],
                        op0=OP.is_lt, op1=OP.max)
                    # e = y·rσ_j (fp16); e2 = e − 1024·drop
                    e = workp.tile([P, TN], F16, tag="w3", name="e")
                    nc.gpsimd.tensor_tensor(
                        e[:], yt_t, rs_b[:, ntb * TN:(ntb + 1) * TN], OP.mult)
                    e2 = workp.tile([P, TN], F16, tag="w1", name="e2")
                    nc.vector.scalar_tensor_tensor(
                        e2[:], drop[:], -DROP_SHIFT, e[:],
                        op0=OP.mult, op1=OP.add)
                    # Wn = exp(rσ_i·e2), written over the f32 y strip,
                    # accum → ΣWn
                    nc.scalar.activation(yt_t, e2[:], ACT.Exp,
                                         scale=rs_cols[:, mt:mt + 1],
                                         accum_out=srcols[:, mt * NNT + ntb:
                                                          mt * NNT + ntb + 1])

                # S = ΣWn + 1 ;  invS = 1/S
                nc.vector.reduce_sum(Scol[:, mt:mt + 1],
                                     srcols[:, mt * NNT:(mt + 1) * NNT],
                                     axis=AX.X)
                nc.vector.tensor_scalar(Scol[:, mt:mt + 1], Scol[:, mt:mt + 1],
                                        1.0, None, OP.add)
                nc.vector.reciprocal(invS[:, mt:mt + 1], Scol[:, mt:mt + 1])
                # Q = offdiag(P) = (Wn − dsel·I)/S  (diagonal handled exactly
                # via d = 2/S in the GEMM correction terms)
                for ntb in range(NNT):
                    dslice = ys[:, ntb * TN + mt * P: ntb * TN + (mt + 1) * P]
                    nc.vector.scalar_tensor_tensor(
                        dslice, ident16[:], ndsel_c[:, ntb:ntb + 1], dslice,
                        op0=OP.mult, op1=OP.add)
                    nc.vector.tensor_scalar(ys[:, ntb * TN:(ntb + 1) * TN],
                                            ys[:, ntb * TN:(ntb + 1) * TN],
                                            invS[:, mt:mt + 1], None, OP.mult)
                    # C1 partial: Σ Q·pen = Σ Wn·pen/S  (pen_ii = 0)
                    pen = workp.tile([P, TN], F16, tag="w3", name="pen")
                    nc.scalar.activation(pen[:], s_b[:, ntb * TN:(ntb + 1) * TN],
                                         ACT.Relu, bias=s_c[:, mt:mt + 1],
                                         scale=-1.0)
                    junk = workp.tile([P, TN], F16, tag="w1", name="junk")
                    nc.vector.scalar_tensor_tensor(
                        junk[:], ys[:, ntb * TN:(ntb + 1) * TN], 1.0, pen[:],
                        op0=OP.mult, op1=OP.mult,
                        accum_out=c1cols[:, mt * NNT + ntb:
                                         mt * NNT + ntb + 1])
                nc.scalar.activation(pn8[mt][:], ys, ACT.Copy)
                nc.sync.dma_start(
                    bass.AP(pn_half[mt], 0, [[B, P], [1, B]]), pn8[mt][:])
                nc.gpsimd.collective_compute(
                    "AllGather", OP.bypass, replica_groups=rg,
                    ins=[pn_half[mt].ap().opt()],
                    outs=[pfull_h[mt].ap().opt()])

            # C1 finalize (1/S already folded in via the Q strip)
            c1v = colp.tile([P, 1], F32, tag="c1v")
            nc.vector.reduce_sum(c1v[:], c1cols[:], axis=AX.X)

            # ============ all-gather of diag d = 2/S (f32) ==================
            dloc = colp.tile([P, NMT], F32, tag="dloc")
            nc.vector.tensor_scalar(dloc[:], invS[:], 2.0, None, OP.mult)
            nc.sync.dma_start(bass.AP(d_dram, 0, [[1, P], [P, NMT]]), dloc[:])
            nc.gpsimd.collective_compute(
                "AllGather", OP.bypass, replica_groups=rg,
                ins=[d_dram.ap().opt()], outs=[d_out.ap().opt()])
            # d in column layout over full B, d² gated by the diag one-hot,
            # and shard d broadcast along the free axis
            d_cols = colp.tile([P, KC], F32, tag="d_cols")
            nc.sync.dma_start(d_cols[:], bass.AP(d_out, 0, [[1, P], [P, KC]]))
            d2g = colp.tile([P, KC], F32, tag="d2g")
            nc.vector.tensor_tensor(d2g[:], d_cols[:], d_cols[:], OP.mult)
            for ntb in range(NNT):
                nc.vector.tensor_scalar(
                    d2g[:, ntb * NMT:(ntb + 1) * NMT],
                    d2g[:, ntb * NMT:(ntb + 1) * NMT],
                    dsel_c[:, ntb:ntb + 1], None, OP.mult)
            d_strip = constp.tile([P, R], F32, tag="d_strip")
            nc.sync.dma_start(d_strip[:], bass.AP(d_dram, 0, [[0, P], [1, R]]))

            # ============ P_shardᵀ (fp8, SBUF) via PE transposes ============
            # transpose the f32 P strips (2 cyc/row), cast fp8 on psum drain
            idf32 = constp.tile([P, P], F32, tag="idf32")
            make_identity(nc, idf32[:])
            psT = bigp.tile([P, KC * R], F8, tag="ptbuf")
            for ntb in range(NNT):
                for u in range(4):
                    ptb = ps_tile(f"pa{4 + (ntb * 4 + u) % 4}", dtype=F32)
                    for mt in range(NMT):
                        nc.tensor.transpose(
                            ptb[:, mt * P:(mt + 1) * P],
                            y_all[:, mt * B + ntb * TN + u * P:
                                  mt * B + ntb * TN + (u + 1) * P],
                            idf32[:])
                    nc.scalar.activation(
                        psT[:, (ntb * 4 + u) * R:(ntb * 4 + u + 1) * R],
                        ptb[:], ACT.Copy)

            # ============ GEMM1: Aᵀ chunks = (P²)ᵀ, consume C2, cast fp8 ====
            c2cols = colp.tile([P, KC], F32, tag="c2cols")
            c3cols = colp.tile([P, KC], F32, tag="c3cols")
            aT = bigp.tile([P, KC * R], F8, tag="atbuf")

            def lhs_strip_load(cbi, phase):
                # ls[p, j, m] with global row-block j = c*4 + q; half h=q//2
                # holds j ≡ 2h, 2h+1 (mod 4) at rows c*256 + (q%2)*128 + p.
                ls = iop.tile([P, KC * P], F8, tag="lhs_strip",
                              name=f"ls{phase}_{cbi}")
                ls4 = ls[:].rearrange("p (c u m) -> p c u m", c=NC, u=4)
                for h in range(4):
                    nc.sync.dma_start(
                        ls4[:, :, h, :],
                        bass.AP(pfull_h[h], cbi * P,
                                [[B, P], [P * B, NC], [1, P]]))
                return ls

            def pen_t_tile(cbi):
                # penᵀ chunk [g-block, i]: relu(s_i − s_g)
                pt = workp.tile([P, TN], F16, tag="w3", name=f"pent{cbi}")
                nc.scalar.activation(pt[:], smy_b[:], ACT.Relu,
                                     bias=sfc[:, cbi:cbi + 1], scale=1.0)
                return pt

            for cb in range(KC):
                ls = lhs_strip_load(cb, 0)
                pa = ps_tile(f"pa{cb % 4}")
                for kb in range(KC2):
                    nc.tensor.matmul(
                        pa[:],
                        ls[:, 2 * kb * P:(2 * kb + 2) * P]
                        .rearrange("p (k m) -> p k m", k=2),
                        psT[:, 2 * kb * R:(2 * kb + 2) * R]
                        .rearrange("p (k n) -> p k n", k=2),
                        start=(kb == 0), stop=(kb == KC2 - 1), perf_mode=DRM)
                # diagonal corrections:
                # Aᵀ = (Q8²)ᵀ + d_g'·Q8ᵀ + Q8ᵀ·d_i + dsel·diag(d²)
                qt = psT[:, cb * R:(cb + 1) * R]
                nc.vector.scalar_tensor_tensor(
                    pa[:], qt, d_cols[:, cb:cb + 1], pa[:],
                    op0=OP.mult, op1=OP.add)
                tdi = workp.tile([P, TN], F32, tag="sqscr", name="tdi")
                nc.vector.tensor_tensor(tdi[:], qt, d_strip[:], OP.mult)
                nc.vector.tensor_tensor(pa[:], pa[:], tdi[:], OP.add)
                dgo = (cb % NMT) * P
                nc.vector.scalar_tensor_tensor(
                    pa[:, dgo:dgo + P], ident16[:], d2g[:, cb:cb + 1],
                    pa[:, dgo:dgo + P], op0=OP.mult, op1=OP.add)
                pent = pen_t_tile(cb)
                junk = workp.tile([P, TN], F16, tag="w1", name="junkA")
                nc.vector.scalar_tensor_tensor(
                    junk[:], pa[:], 1.0, pent[:], op0=OP.mult, op1=OP.mult,
                    accum_out=c2cols[:, cb:cb + 1])
                nc.scalar.activation(aT[:, cb * R:(cb + 1) * R], pa[:],
                                     ACT.Copy)

            # ============ GEMM2: Uᵀ chunks = (A·P)ᵀ, consume C3 =============
            for cb in range(KC):
                ls = lhs_strip_load(cb, 1)
                pa = ps_tile(f"pa{4 + cb % 4}")
                for kb in range(KC2):
                    nc.tensor.matmul(
                        pa[:],
                        ls[:, 2 * kb * P:(2 * kb + 2) * P]
                        .rearrange("p (k m) -> p k m", k=2),
                        aT[:, 2 * kb * R:(2 * kb + 2) * R]
                        .rearrange("p (k n) -> p k n", k=2),
                        start=(kb == 0), stop=(kb == KC2 - 1), perf_mode=DRM)
                # Uᵀ = (A·Q8)ᵀ + d_g'·Aᵀ
                nc.vector.scalar_tensor_tensor(
                    pa[:], aT[:, cb * R:(cb + 1) * R], d_cols[:, cb:cb + 1],
                    pa[:], op0=OP.mult, op1=OP.add)
                pent = pen_t_tile(cb)
                junk = workp.tile([P, TN], F16, tag="w1", name="junkU")
                nc.vector.scalar_tensor_tensor(
                    junk[:], pa[:], 1.0, pent[:], op0=OP.mult, op1=OP.mult,
                    accum_out=c3cols[:, cb:cb + 1])

            # ============ final reduction ==================================
            c2v = colp.tile([P, 1], F32, tag="c2v")
            c3v = colp.tile([P, 1], F32, tag="c3v")
            nc.vector.reduce_sum(c2v[:], c2cols[:], axis=AX.X)
            nc.vector.reduce_sum(c3v[:], c3cols[:], axis=AX.X)
            tot = colp.tile([P, 1], F32, tag="tot")
            nc.vector.tensor_scalar(tot[:], c2v[:], 0.5, None, OP.mult)
            nc.vector.tensor_tensor(tot[:], tot[:], c1v[:], OP.add)
            nc.vector.tensor_scalar(c3v[:], c3v[:], 1.0 / 3.0, None, OP.mult)
            nc.vector.tensor_tensor(tot[:], tot[:], c3v[:], OP.add)

            ones_c = constp.tile([P, 1], F32, tag="ones_c")
            nc.vector.memset(ones_c[:], 1.0)
            fin = ps_tile("pa0", shape=[1, 8])
            nc.tensor.matmul(fin[:, 0:1], tot[:], ones_c[:], start=True,
                             stop=True)
            lsb = colp.tile([1, 8], F32, tag="lsb")
            nc.vector.memset(lsb[:], 0.0)
            nc.scalar.activation(lsb[:, 0:1], fin[:, 0:1], ACT.Copy,
                                 scale=1.0 / float(B))
            nc.sync.dma_start(red_in[:, :], lsb[:])
            nc.gpsimd.collective_compute(
                "AllReduce", OP.add, replica_groups=rg,
                ins=[red_in.ap().opt()], outs=[red_out.ap().opt()])
            nc.sync.dma_start(loss_out[:, :], red_out[0:1, 0:1])

    nc.compile()
    return nc


def make_inputs(features, scores, labels, B, D, NC):
    """Build the per-core input maps from full inputs."""
    R = B // NC
    P = 128
    NMT = R // P
    NNT = B // R
    f = np.ascontiguousarray(features, dtype=np.float32)
    s = np.ascontiguousarray(scores, dtype=np.float32).reshape(B)
    lab = np.asarray(labels).astype(np.float32).reshape(B)
    ftf = np.ascontiguousarray(f.T)
    in_maps = []
    for c in range(NC):
        sh = slice(c * R, (c + 1) * R)
        onehot = np.zeros((1, NNT), dtype=np.float32)
        onehot[0, c] = 1.0
        in_maps.append({
            "ft2": np.ascontiguousarray(2.0 * f[sh].T).astype(ml_dtypes.bfloat16),
            "ftf": ftf.astype(ml_dtypes.bfloat16),
            "fsh": np.ascontiguousarray(f[sh]).astype(ml_dtypes.bfloat16),
            "srow": s.reshape(1, B),
            "srow16": s.reshape(1, B).astype(np.float16),
            "smyrow16": np.ascontiguousarray(s[sh]).reshape(1, R)
            .astype(np.float16),
            "scols": np.ascontiguousarray(s[sh].reshape(NMT, P).T),
            "lrow16": lab.reshape(1, B).astype(np.float16),
            "lcols": np.ascontiguousarray(lab[sh].reshape(NMT, P).T),
            "dsel": onehot,
        })
    return in_maps


_cached = {}


def kernel(features, scores, labels):
    B, D = features.shape
    NC = 8
    key = (B, D)
    if key not in _cached:
        _cached[key] = build_program(B=B, D=D, NC=NC)
    nc = _cached[key]
    from concourse.bass_utils import run_bass_kernel_spmd
    in_maps = make_inputs(features, scores, labels, B, D, NC)
    res = run_bass_kernel_spmd(nc, in_maps, core_ids=list(range(NC)))
    out = res.results[0]["loss"]
    return np.float32(out.reshape(())[()])


# revision 47
# speedup vs baseline: 1.3640x; 1.1657x over previous
"""Trainium2 Bass kernel for nn_DirectedODRLoss (retrieval_knn).

Math (B=4096, D=256, k=25, scales (1,2,3)):
    dist²(i,j) = |f_i|² + |f_j|² − 2 f_i·f_j ;  y := −dist²  (f32 strips;
        bf16 Gram GEMM; −|f_i|² per-partition scalar and −|f_j|² f32 row
        strip folded into one DVE scalar_tensor_tensor on the PSUM drain)
    topk: per row, candidates = top-8 of each of 16 256-wide chunks (16
        f32 max8 passes) → top-25 of the 128 candidates via max8 +
        match_replace;  τ_i := 25th largest y;  σ_i = mean(sqrt(−y+eps))
    mutual knn mask:  y symmetric ⇒ mutual(i,j) = [y_ij ≥ max(τ_i, τ_j)]
    drop = [max(τ_i,τ_j) > y] | [lab_j < lab_i];  e = y·rσ_j (fp16)
    Wn = exp(rσ_i·(e − 1024·drop))   (drop ⇒ exp(≤ −45/σ²) ⇒ 0 in fp16)
    S_i = ΣWn + 1,  P = (Wn + dsel·I)/S  (f32 in place; diag via a
        per-core one-hot dsel over column blocks), quantized to fp8 e4m3
    loss = (1/B)(C1 + C2/2 + C3/3) with
        C1 = <Wn/S, pen>,  C2 = <A, pen>,  C3 = <U, pen>,
        A = P²  (computed transposed: Aᵀ chunks, lhsT = raw pfull column
        strips, rhs = SBUF-resident P_shardᵀ),  U = A·P (Uᵀ likewise),
        pen_ij = relu(s_i − s_j) (fp16).

Sharding: rows split across 8 cores. P all-gathered in fp8; both B³ GEMMs
run in fp8 DoubleRow mode (0.5 cyc/row); Aᵀ/Uᵀ chunks consumed against
recomputed penᵀ tiles via fused scalar_tensor_tensor accumulations.
Final scalars all-reduced.
"""

import ml_dtypes
import numpy as np

import concourse.bacc as bacc
import concourse.bass as bass
import concourse.mybir as mybir
import concourse.tile as tile
from concourse.masks import make_identity

F32 = mybir.dt.float32
F16 = mybir.dt.float16
BF16 = mybir.dt.bfloat16
F8 = mybir.dt.float8e4
AX = mybir.AxisListType
OP = mybir.AluOpType
ACT = mybir.ActivationFunctionType
DRM = mybir.MatmulPerfMode.DoubleRow

EPS = 1e-8
KNN = 25
DROP_SHIFT = 1024.0
NEG_FILL = -1e30


def build_program(B=4096, D=256, NC=8):
    P = 128
    R = B // NC            # rows per core (512)
    NMT = R // P           # row tiles per core (4)
    TN = R                 # column tile (512)
    NNT = B // TN          # column tiles (8)
    KC = B // P            # 128-row chunks of B (32)
    KC2 = KC // 2          # DoubleRow steps (16)
    GK = D // P            # contraction chunks for the Gram GEMM (2)
    TC = 16                # topk candidate chunks per row
    CW = B // TC           # chunk width (256)

    nc = bacc.Bacc("TRN2", target_bir_lowering=False, debug=False,
                   num_devices=NC)

    # ---- I/O ----------------------------------------------------------------
    ft2 = nc.dram_tensor("ft2", [D, R], BF16, kind="ExternalInput")    # 2·F_shardᵀ
    ftf = nc.dram_tensor("ftf", [D, B], BF16, kind="ExternalInput")    # Fᵀ (full)
    fsh = nc.dram_tensor("fsh", [R, D], BF16, kind="ExternalInput")    # F shard
    srow = nc.dram_tensor("srow", [1, B], F32, kind="ExternalInput")   # scores
    srow16 = nc.dram_tensor("srow16", [1, B], F16, kind="ExternalInput")
    smyrow16 = nc.dram_tensor("smyrow16", [1, R], F16, kind="ExternalInput")
    scols = nc.dram_tensor("scols", [P, NMT], F32, kind="ExternalInput")
    lrow16 = nc.dram_tensor("lrow16", [1, B], F16, kind="ExternalInput")
    lcols = nc.dram_tensor("lcols", [P, NMT], F32, kind="ExternalInput")
    dsel = nc.dram_tensor("dsel", [1, NNT], F32, kind="ExternalInput")  # one-hot(rank)
    loss_out = nc.dram_tensor("loss", [1, 1], F32, kind="ExternalOutput")

    NMT0 = (B // NC) // 128
    # ---- internal DRAM ------------------------------------------------------
    # P is all-gathered in four row-quarters (one per mt tile) so the AGs
    # overlap stage W. Quarter q of pfull holds rank-major 128-row blocks:
    # global row c*R + q*128 + p ↔ pfull_q[c*128 + p].
    pn_half = [nc.dram_tensor(f"pn_half{h}", [P, B], F8) for h in range(NMT0)]
    pfull_h = [nc.dram_tensor(f"pfull{h}", [NC * P, B], F8,
                              addr_space="Shared") for h in range(NMT0)]
    stats_in = nc.dram_tensor("stats_in", [1, 2 * R], F32)
    stats_out = nc.dram_tensor("stats_out", [NC, 2 * R], F32, addr_space="Shared")
    d_dram = nc.dram_tensor("d_dram", [1, R], F32)
    d_out = nc.dram_tensor("d_out", [NC, R], F32, addr_space="Shared")
    red_in = nc.dram_tensor("red_in", [1, 8], F32)
    red_out = nc.dram_tensor("red_out", [1, 8], F32, addr_space="Shared")

    rg = [list(range(NC))]

    with tile.TileContext(nc) as tc:
        with (
            tc.tile_pool(name="const", bufs=1) as constp,
            tc.tile_pool(name="io", bufs=2) as iop,
            tc.tile_pool(name="big", bufs=1) as bigp,
            tc.tile_pool(name="strip", bufs=1) as stripp,
            tc.tile_pool(name="cols", bufs=1) as colp,
            tc.tile_pool(name="work", bufs=2) as workp,
            tc.tile_pool(name="psum", bufs=1, space="PSUM") as psump,
        ):
            def ps_tile(tag, shape=None, dtype=F32):
                return psump.tile(shape or [P, TN], dtype, tag=tag, name=tag)

            # Gram operands (bf16, cached in SBUF, reused per mt tile)
            ft2_sb = constp.tile([P, GK * R], BF16, tag="ft2_sb")
            for g in range(GK):
                nc.sync.dma_start(ft2_sb[:, g * R:(g + 1) * R],
                                  ft2[g * P:(g + 1) * P, :])
            ftf_sb = bigp.tile([P, GK * B], BF16, tag="fbuf", name="ftf_sb")
            for g in range(GK):
                nc.sync.dma_start(ftf_sb[:, g * B:(g + 1) * B],
                                  ftf[g * P:(g + 1) * P, :])

            # ============ stage A: squared norms ============================
            # full |f|²: square ftf (f32 out), column-reduce via ones-matmul
            # broadcast to all 128 partitions (lhsT = all-ones matrix)
            sq2 = bigp.tile([P, GK * B], F32, tag="ybuf", name="sq2")
            for g in range(GK):
                nc.scalar.activation(sq2[:, g * B:(g + 1) * B],
                                     ftf_sb[:, g * B:(g + 1) * B], ACT.Square)
            ones_m = constp.tile([P, P], F32, tag="ones_m")
            nc.vector.memset(ones_m[:], 1.0)
            sqj_b = stripp.tile([P, B], F32, tag="sqj_b")
            for ntb in range(NNT):
                psq = ps_tile(f"pa{ntb % 4}")
                for g in range(GK):
                    nc.tensor.matmul(
                        psq[:], ones_m[:],
                        sq2[:, g * B + ntb * TN: g * B + (ntb + 1) * TN],
                        start=(g == 0), stop=(g == GK - 1))
                nc.scalar.activation(sqj_b[:, ntb * TN:(ntb + 1) * TN],
                                     psq[:], ACT.Copy)
            # shard |f|² in column layout (per-partition scalars, negated)
            sqcs = colp.tile([P, NMT], F32, tag="sqcs")
            fshb = iop.tile([P, NMT * D], BF16, tag="fblk", name="fshb")
            nc.sync.dma_start(
                fshb[:].rearrange("p (t d) -> p t d", t=NMT),
                bass.AP(fsh, 0, [[D, P], [P * D, NMT], [1, D]]))
            for q in range(NMT):
                scr = workp.tile([P, D], F32, tag="sqscr")
                nc.scalar.activation(scr[:], fshb[:, q * D:(q + 1) * D],
                                     ACT.Square, accum_out=sqcs[:, q:q + 1])
            sqcsr = colp.tile([P, NMT], F32, tag="sqcsr")
            nc.vector.tensor_scalar(sqcsr[:], sqcs[:], -1.0, None, OP.mult)

            # misc broadcast loads
            dsel_c = colp.tile([P, NNT], F32, tag="dsel_c")
            nc.sync.dma_start(dsel_c[:], bass.AP(dsel, 0, [[0, P], [1, NNT]]))
            lab_c = colp.tile([P, NMT], F32, tag="lab_c")
            s_c = colp.tile([P, NMT], F32, tag="s_c")
            nc.sync.dma_start(lab_c[:], lcols[:, :])
            nc.sync.dma_start(s_c[:], scols[:, :])
            lab_b = stripp.tile([P, B], F16, tag="lab_b")
            s_b = stripp.tile([P, B], F16, tag="s_b")
            smy_b = constp.tile([P, R], F16, tag="smy_b")
            nc.sync.dma_start(lab_b[:], bass.AP(lrow16, 0, [[0, P], [1, B]]))
            nc.sync.dma_start(s_b[:], bass.AP(srow16, 0, [[0, P], [1, B]]))
            nc.sync.dma_start(smy_b[:], bass.AP(smyrow16, 0, [[0, P], [1, R]]))
            sfc = colp.tile([P, KC], F32, tag="sfc")       # −s_g, col layout
            nc.sync.dma_start(sfc[:], bass.AP(srow, 0, [[1, P], [P, KC]]))
            nc.vector.tensor_scalar(sfc[:], sfc[:], -1.0, None, OP.mult)

            ident16 = constp.tile([P, P], F16, tag="ident16")
            make_identity(nc, ident16[:])
            ndsel_c = colp.tile([P, NNT], F32, tag="ndsel_c")
            nc.vector.tensor_scalar(ndsel_c[:], dsel_c[:], -1.0, None, OP.mult)
            eps_c = constp.tile([P, 1], F32, tag="eps_c")
            nc.vector.memset(eps_c[:], EPS)

            # ============ stage B: Gram → y (f32) + chunked topk ============
            y_all = bigp.tile([P, NMT * B], F32, tag="ybuf", name="y_all")
            cands = colp.tile([P, TC * 8], F32, tag="cands")
            vals = colp.tile([P, 32], F32, tag="vals")
            yt_cols = colp.tile([P, NMT], F32, tag="yt_cols")
            rs_cols = colp.tile([P, NMT], F32, tag="rs_cols")
            ssum = colp.tile([P, NMT], F32, tag="ssum")
            for mt in range(NMT):
                gps = [ps_tile(f"pa{ntb}") for ntb in range(NNT)]
                for g in range(GK):
                    for ntb in range(NNT):
                        nc.tensor.matmul(
                            gps[ntb][:],
                            ft2_sb[:, g * R + mt * P: g * R + (mt + 1) * P],
                            ftf_sb[:, g * B + ntb * TN: g * B + (ntb + 1) * TN],
                            start=(g == 0), stop=(g == GK - 1))
                ys = y_all[:, mt * B:(mt + 1) * B]
                for ntb in range(NNT):
                    # y = (gram − |f_i|²) − |f_j|²   (f32 out)
                    nc.vector.scalar_tensor_tensor(
                        ys[:, ntb * TN:(ntb + 1) * TN], gps[ntb][:],
                        sqcsr[:, mt:mt + 1], sqj_b[:, ntb * TN:(ntb + 1) * TN],
                        op0=OP.add, op1=OP.subtract)
                # candidates: top-8 of each 256-wide chunk
                for t in range(TC):
                    nc.vector.max(out=cands[:, t * 8:(t + 1) * 8],
                                  in_=ys[:, t * CW:(t + 1) * CW])
                # top-25 of the 128 candidates
                ca = workp.tile([P, TC * 8], F32, tag="ca", name="ca")
                cb = workp.tile([P, TC * 8], F32, tag="cb", name="cb")
                nc.vector.max(out=vals[:, 0:8], in_=cands[:])
                nc.vector.match_replace(out=ca[:], in_to_replace=vals[:, 0:8],
                                        in_values=cands[:], imm_value=NEG_FILL)
                nc.vector.max(out=vals[:, 8:16], in_=ca[:])
                nc.vector.match_replace(out=cb[:], in_to_replace=vals[:, 8:16],
                                        in_values=ca[:], imm_value=NEG_FILL)
                nc.vector.max(out=vals[:, 16:24], in_=cb[:])
                nc.vector.match_replace(out=ca[:], in_to_replace=vals[:, 16:24],
                                        in_values=cb[:], imm_value=NEG_FILL)
                nc.vector.max(out=vals[:, 24:32], in_=ca[:])
                # τ_i = 25th largest y
                nc.vector.tensor_copy(yt_cols[:, mt:mt + 1], vals[:, 24:25])
                # σ_i = mean sqrt(max(d,0)+eps) over 25 NN;  d = −y
                c25 = workp.tile([P, KNN], F32, tag="c25")
                nc.vector.tensor_scalar(c25[:], vals[:, 0:KNN], 0.0, None,
                                        OP.min)
                s25 = workp.tile([P, KNN], F32, tag="s25")
                nc.scalar.activation(s25[:], c25[:], ACT.Sqrt,
                                     bias=eps_c[:, 0:1], scale=-1.0,
                                     accum_out=ssum[:, mt:mt + 1])
            nc.vector.reciprocal(rs_cols[:], ssum[:])
            nc.vector.tensor_scalar(rs_cols[:], rs_cols[:], float(KNN), None,
                                    OP.mult)

            # stats all-gather: flat per-rank [τ(R) ++ rσ(R)] (f32), both in
            # shard-row order g_local = c*128 + p.
            nc.sync.dma_start(bass.AP(stats_in, 0, [[1, P], [P, NMT]]),
                              yt_cols[:])
            nc.sync.dma_start(bass.AP(stats_in, R, [[1, P], [P, NMT]]),
                              rs_cols[:])
            nc.gpsimd.collective_compute(
                "AllGather", OP.bypass, replica_groups=rg,
                ins=[stats_in.ap().opt()], outs=[stats_out.ap().opt()])

            def stat_bcast_ap(off):
                return bass.AP(stats_out, off, [[0, P], [2 * R, NC], [1, R]])

            yt_b = stripp.tile([P, B], F32, tag="yt_b")
            nc.sync.dma_start(yt_b[:].rearrange("a (r q) -> a r q", r=NC),
                              stat_bcast_ap(0))
            # rσ strip: stage f32 into sqj_b's buffer (dead after stage B),
            # convert to fp16
            rs_f32 = stripp.tile([P, B], F32, tag="sqj_b", name="rs_f32")
            nc.sync.dma_start(rs_f32[:].rearrange("a (r q) -> a r q", r=NC),
                              stat_bcast_ap(R))
            rs_b = stripp.tile([P, B], F16, tag="rs_b")
            nc.vector.tensor_copy(rs_b[:], rs_f32[:])

            # ============ stage W: Wn, S, P, C1 =============================
            srcols = colp.tile([P, NMT * NNT], F32, tag="srcols")
            c1cols = colp.tile([P, NMT * NNT], F32, tag="c1cols")
            invS = colp.tile([P, NMT], F32, tag="invS")
            Scol = colp.tile([P, NMT], F32, tag="Scol")
            def w_phase1(mt):
                ys = y_all[:, mt * B:(mt + 1) * B]
                for ntb in range(NNT):
                    yt_t = ys[:, ntb * TN:(ntb + 1) * TN]
                    # drop = [max(τ_j, τ_i) > y] | [lab_j < lab_i]
                    drop1 = workp.tile([P, TN], F16, tag="w1", name="drop1")
                    nc.vector.scalar_tensor_tensor(
                        drop1[:], yt_b[:, ntb * TN:(ntb + 1) * TN],
                        yt_cols[:, mt:mt + 1], yt_t,
                        op0=OP.max, op1=OP.is_gt)
                    drop = workp.tile([P, TN], F16, tag="w2", name="drop")
                    nc.vector.scalar_tensor_tensor(
                        drop[:], lab_b[:, ntb * TN:(ntb + 1) * TN],
                        lab_c[:, mt:mt + 1], drop1[:],
                        op0=OP.is_lt, op1=OP.max)
                    # e = y·rσ_j (fp16); e2 = e − 1024·drop
                    e = workp.tile([P, TN], F16, tag="w3", name="e")
                    nc.gpsimd.tensor_tensor(
                        e[:], yt_t, rs_b[:, ntb * TN:(ntb + 1) * TN], OP.mult)
                    e2 = workp.tile([P, TN], F16, tag="w1", name="e2")
                    nc.vector.scalar_tensor_tensor(
                        e2[:], drop[:], -DROP_SHIFT, e[:],
                        op0=OP.mult, op1=OP.add)
                    # Wn = exp(rσ_i·e2), over the f32 y strip, accum → ΣWn
                    nc.scalar.activation(yt_t, e2[:], ACT.Exp,
                                         scale=rs_cols[:, mt:mt + 1],
                                         accum_out=srcols[:, mt * NNT + ntb:
                                                          mt * NNT + ntb + 1])

            def w_phase2(mt):
                ys = y_all[:, mt * B:(mt + 1) * B]
                # S = ΣWn + 1 ;  invS = 1/S
                nc.vector.reduce_sum(Scol[:, mt:mt + 1],
                                     srcols[:, mt * NNT:(mt + 1) * NNT],
                                     axis=AX.X)
                nc.vector.tensor_scalar(Scol[:, mt:mt + 1], Scol[:, mt:mt + 1],
                                        1.0, None, OP.add)
                nc.vector.reciprocal(invS[:, mt:mt + 1], Scol[:, mt:mt + 1])
                # Q = offdiag(P) = (Wn − dsel·I)/S  (diag exact via d = 2/S)
                for ntb in range(NNT):
                    dslice = ys[:, ntb * TN + mt * P: ntb * TN + (mt + 1) * P]
                    nc.vector.scalar_tensor_tensor(
                        dslice, ident16[:], ndsel_c[:, ntb:ntb + 1], dslice,
                        op0=OP.mult, op1=OP.add)
                    nc.vector.tensor_scalar(ys[:, ntb * TN:(ntb + 1) * TN],
                                            ys[:, ntb * TN:(ntb + 1) * TN],
                                            invS[:, mt:mt + 1], None, OP.mult)
                    # C1 partial: Σ Q·pen = Σ Wn·pen/S  (pen_ii = 0)
                    pen = workp.tile([P, TN], F16, tag="w3", name="pen")
                    nc.scalar.activation(pen[:], s_b[:, ntb * TN:(ntb + 1) * TN],
                                         ACT.Relu, bias=s_c[:, mt:mt + 1],
                                         scale=-1.0)
                    junk = workp.tile([P, TN], F16, tag="w1", name="junk")
                    nc.vector.scalar_tensor_tensor(
                        junk[:], ys[:, ntb * TN:(ntb + 1) * TN], 1.0, pen[:],
                        op0=OP.mult, op1=OP.mult,
                        accum_out=c1cols[:, mt * NNT + ntb:
                                         mt * NNT + ntb + 1])
                for ntb in range(NNT):
                    p8t = workp.tile([P, TN], F8, tag="pn8t", name="p8t")
                    nc.scalar.activation(p8t[:],
                                         ys[:, ntb * TN:(ntb + 1) * TN],
                                         ACT.Copy)
                    nc.sync.dma_start(
                        bass.AP(pn_half[mt], ntb * TN, [[B, P], [1, TN]]),
                        p8t[:])
                nc.gpsimd.collective_compute(
                    "AllGather", OP.bypass, replica_groups=rg,
                    ins=[pn_half[mt].ap().opt()],
                    outs=[pfull_h[mt].ap().opt()])

            # software pipeline: phase1(mt+1) is emitted before phase2(mt)
            # so DVE never head-of-line blocks on the scalar-engine exps
            w_phase1(0)
            w_phase1(1)
            w_phase2(0)
            w_phase1(2)
            w_phase2(1)
            w_phase1(3)
            w_phase2(2)
            w_phase2(3)

            # C1 finalize (1/S already folded in via the Q strip)
            c1v = colp.tile([P, 1], F32, tag="c1v")
            nc.vector.reduce_sum(c1v[:], c1cols[:], axis=AX.X)

            # ============ all-gather of diag d = 2/S (f32) ==================
            dloc = colp.tile([P, NMT], F32, tag="dloc")
            nc.vector.tensor_scalar(dloc[:], invS[:], 2.0, None, OP.mult)
            nc.sync.dma_start(bass.AP(d_dram, 0, [[1, P], [P, NMT]]), dloc[:])
            nc.gpsimd.collective_compute(
                "AllGather", OP.bypass, replica_groups=rg,
                ins=[d_dram.ap().opt()], outs=[d_out.ap().opt()])
            # d in column layout over full B, d² gated by the diag one-hot,
            # and shard d broadcast along the free axis
            d_cols = colp.tile([P, KC], F32, tag="d_cols")
            nc.sync.dma_start(d_cols[:], bass.AP(d_out, 0, [[1, P], [P, KC]]))
            d2g = colp.tile([P, KC], F32, tag="d2g")
            nc.vector.tensor_tensor(d2g[:], d_cols[:], d_cols[:], OP.mult)
            for ntb in range(NNT):
                nc.vector.tensor_scalar(
                    d2g[:, ntb * NMT:(ntb + 1) * NMT],
                    d2g[:, ntb * NMT:(ntb + 1) * NMT],
                    dsel_c[:, ntb:ntb + 1], None, OP.mult)
            d_strip = constp.tile([P, R], F32, tag="d_strip")
            nc.sync.dma_start(d_strip[:], bass.AP(d_dram, 0, [[0, P], [1, R]]))

            # ============ P_shardᵀ (fp8, SBUF) via PE transposes ============
            # transpose the f32 P strips (2 cyc/row), cast fp8 on psum drain
            idf32 = constp.tile([P, P], F32, tag="idf32")
            make_identity(nc, idf32[:])
            psT = bigp.tile([P, KC * R], F8, tag="ptbuf")
            for ntb in range(NNT):
                for u in range(4):
                    ptb = ps_tile(f"pa{4 + (ntb * 4 + u) % 4}", dtype=F32)
                    for mt in range(NMT):
                        nc.tensor.transpose(
                            ptb[:, mt * P:(mt + 1) * P],
                            y_all[:, mt * B + ntb * TN + u * P:
                                  mt * B + ntb * TN + (u + 1) * P],
                            idf32[:])
                    nc.scalar.activation(
                        psT[:, (ntb * 4 + u) * R:(ntb * 4 + u + 1) * R],
                        ptb[:], ACT.Copy)

            # ============ GEMM1: Aᵀ chunks = (P²)ᵀ, consume C2, cast fp8 ====
            c2cols = colp.tile([P, KC], F32, tag="c2cols")
            c3cols = colp.tile([P, KC], F32, tag="c3cols")
            aT = bigp.tile([P, KC * R], F8, tag="atbuf")

            def lhs_strip_load(cb2, phase):
                # two output chunks per load; layout [p, u(quarter), c, m2]
                ls = iop.tile([P, 4 * NC * 2 * P], F8, tag="lhs_strip",
                              name=f"ls{phase}_{cb2}")
                ls4 = ls[:].rearrange("p (u c m) -> p u c m", u=4, c=NC)
                for h in range(4):
                    nc.sync.dma_start(
                        ls4[:, h, :, :],
                        bass.AP(pfull_h[h], cb2 * 2 * P,
                                [[B, P], [P * B, NC], [1, 2 * P]]))
                return ls

            def pen_t_tile(cbi):
                # penᵀ chunk [g-block, i]: relu(s_i − s_g)
                pt = workp.tile([P, TN], F16, tag="w3", name=f"pent{cbi}")
                nc.scalar.activation(pt[:], smy_b[:], ACT.Relu,
                                     bias=sfc[:, cbi:cbi + 1], scale=1.0)
                return pt

            for cb in range(KC):
                if cb % 2 == 0:
                    ls = lhs_strip_load(cb // 2, 0)
                dlt = cb % 2
                lsr = ls[:].rearrange("p (u c m) -> p u c m", u=4, c=NC)
                pa = ps_tile(f"pa{cb % 4}")
                for kb in range(KC2):
                    cc, up = kb // 2, kb % 2
                    j0 = 4 * cc + 2 * up
                    nc.tensor.matmul(
                        pa[:],
                        lsr[:, 2 * up:2 * up + 2, cc:cc + 1,
                            dlt * P:(dlt + 1) * P],
                        psT[:, j0 * R:(j0 + 2) * R]
                        .rearrange("p (k n) -> p k n", k=2),
                        start=(kb == 0), stop=(kb == KC2 - 1), perf_mode=DRM)
                # diagonal corrections:
                # Aᵀ = (Q8²)ᵀ + d_g'·Q8ᵀ + Q8ᵀ·d_i + dsel·diag(d²)
                qt = psT[:, cb * R:(cb + 1) * R]
                nc.vector.scalar_tensor_tensor(
                    pa[:], qt, d_cols[:, cb:cb + 1], pa[:],
                    op0=OP.mult, op1=OP.add)
                tdi = workp.tile([P, TN], F16, tag="w3", name="tdi")
                nc.vector.tensor_tensor(tdi[:], qt, d_strip[:], OP.mult)
                nc.vector.tensor_tensor(pa[:], pa[:], tdi[:], OP.add)
                dgo = (cb % NMT) * P
                nc.vector.scalar_tensor_tensor(
                    pa[:, dgo:dgo + P], ident16[:], d2g[:, cb:cb + 1],
                    pa[:, dgo:dgo + P], op0=OP.mult, op1=OP.add)
                pent = pen_t_tile(cb)
                junk = workp.tile([P, TN], F16, tag="w1", name="junkA")
                nc.vector.scalar_tensor_tensor(
                    junk[:], pa[:], 1.0, pent[:], op0=OP.mult, op1=OP.mult,
                    accum_out=c2cols[:, cb:cb + 1])
                nc.scalar.activation(aT[:, cb * R:(cb + 1) * R], pa[:],
                                     ACT.Copy)

            # ============ GEMM2: Uᵀ chunks = (A·P)ᵀ, consume C3 =============
            for cb in range(KC):
                if cb % 2 == 0:
                    ls = lhs_strip_load(cb // 2, 1)
                dlt = cb % 2
                lsr = ls[:].rearrange("p (u c m) -> p u c m", u=4, c=NC)
                pa = ps_tile(f"pa{4 + cb % 4}")
                for kb in range(KC2):
                    cc, up = kb // 2, kb % 2
                    j0 = 4 * cc + 2 * up
                    nc.tensor.matmul(
                        pa[:],
                        lsr[:, 2 * up:2 * up + 2, cc:cc + 1,
                            dlt * P:(dlt + 1) * P],
                        aT[:, j0 * R:(j0 + 2) * R]
                        .rearrange("p (k n) -> p k n", k=2),
                        start=(kb == 0), stop=(kb == KC2 - 1), perf_mode=DRM)
                # Uᵀ = (A·Q8)ᵀ + d_g'·Aᵀ
                nc.vector.scalar_tensor_tensor(
                    pa[:], aT[:, cb * R:(cb + 1) * R], d_cols[:, cb:cb + 1],
                    pa[:], op0=OP.mult, op1=OP.add)
                pent = pen_t_tile(cb)
                junk = workp.tile([P, TN], F16, tag="w1", name="junkU")
                nc.vector.scalar_tensor_tensor(
                    junk[:], pa[:], 1.0, pent[:], op0=OP.mult, op1=OP.mult,
                    accum_out=c3cols[:, cb:cb + 1])

            # ============ final reduction ==================================
            c2v = colp.tile([P, 1], F32, tag="c2v")
            c3v = colp.tile([P, 1], F32, tag="c3v")
            nc.vector.reduce_sum(c2v[:], c2cols[:], axis=AX.X)
            nc.vector.reduce_sum(c3v[:], c3cols[:], axis=AX.X)
            tot = colp.tile([P, 1], F32, tag="tot")
            nc.vector.tensor_scalar(tot[:], c2v[:], 0.5, None, OP.mult)
            nc.vector.tensor_tensor(tot[:], tot[:], c1v[:], OP.add)
            nc.vector.tensor_scalar(c3v[:], c3v[:], 1.0 / 3.0, None, OP.mult)
            nc.vector.tensor_tensor(tot[:], tot[:], c3v[:], OP.add)

            ones_c = constp.tile([P, 1], F32, tag="ones_c")
            nc.vector.memset(ones_c[:], 1.0)
            fin = ps_tile("pa0", shape=[1, 8])
            nc.tensor.matmul(fin[:, 0:1], tot[:], ones_c[:], start=True,
                             stop=True)
            lsb = colp.tile([1, 8], F32, tag="lsb")
            nc.vector.memset(lsb[:], 0.0)
            nc.scalar.activation(lsb[:, 0:1], fin[:, 0:1], ACT.Copy,
                                 scale=1.0 / float(B))
            nc.sync.dma_start(red_in[:, :], lsb[:])
            nc.gpsimd.collective_compute(
                "AllReduce", OP.add, replica_groups=rg,
                ins=[red_in.ap().opt()], outs=[red_out.ap().opt()])
            nc.sync.dma_start(loss_out[:, :], red_out[0:1, 0:1])

    nc.compile()
    return nc


def make_inputs(features, scores, labels, B, D, NC):
    """Build the per-core input maps from full inputs."""
    R = B // NC
    P = 128
    NMT = R // P
    NNT = B // R
    f = np.ascontiguousarray(features, dtype=np.float32)
    s = np.ascontiguousarray(scores, dtype=np.float32).reshape(B)
    lab = np.asarray(labels).astype(np.float32).reshape(B)
    ftf = np.ascontiguousarray(f.T)
    in_maps = []
    for c in range(NC):
        sh = slice(c * R, (c + 1) * R)
        onehot = np.zeros((1, NNT), dtype=np.float32)
        onehot[0, c] = 1.0
        in_maps.append({
            "ft2": np.ascontiguousarray(2.0 * f[sh].T).astype(ml_dtypes.bfloat16),
            "ftf": ftf.astype(ml_dtypes.bfloat16),
            "fsh": np.ascontiguousarray(f[sh]).astype(ml_dtypes.bfloat16),
            "srow": s.reshape(1, B),
            "srow16": s.reshape(1, B).astype(np.float16),
            "smyrow16": np.ascontiguousarray(s[sh]).reshape(1, R)
            .astype(np.float16),
            "scols": np.ascontiguousarray(s[sh].reshape(NMT, P).T),
            "lrow16": lab.reshape(1, B).astype(np.float16),
            "lcols": np.ascontiguousarray(lab[sh].reshape(NMT, P).T),
            "dsel": onehot,
        })
    return in_maps


_cached = {}


def kernel(features, scores, labels):
    B, D = features.shape
    NC = 8
    key = (B, D)
    if key not in _cached:
        _cached[key] = build_program(B=B, D=D, NC=NC)
    nc = _cached[key]
    from concourse.bass_utils import run_bass_kernel_spmd
    in_maps = make_inputs(features, scores, labels, B, D, NC)
    res = run_bass_kernel_spmd(nc, in_maps, core_ids=list(range(NC)))
    out = res.results[0]["loss"]
    return np.float32(out.reshape(())[()])
